# revision 18
# baseline (speedup 1.0000x reference)
"""AttnRNNCell Trainium2 kernel: 8-core data-parallel over batch.

Feature-major activations [H, B_local]; bf16 matmuls except the q GEMM,
which runs in fp8e4 DoubleRow (2x PE rate; softmax damps the quantization).
The h-only half of the gate GEMM (wg1 = Wg1-Wg2 acting on h) is hoisted into
the softmax/attn-combine window to keep the PE busy, accumulated to SBUF, and
re-injected into the gate PSUM via an identity-stationary matmul.

Gate algebra: sigmoid(Wg1 h + Wg2 attn_o + bg) == sigmoid((Wg1-Wg2) h +
Wg2 (attn_o + h) + bg), so the gate GEMM part2 consumes y = attn_o + h.
"""
import sys

sys.path.insert(0, "/opt/trn_rl_repo")

import numpy as np
import ml_dtypes

import concourse.bass as bass
import concourse.tile as tile
import concourse.mybir as mybir
from concourse.bass_utils import run_bass_kernel_spmd

F32 = mybir.dt.float32
F32R = mybir.dt.float32r
BF16 = mybir.dt.bfloat16
FP8 = mybir.dt.float8e4
AF = mybir.ActivationFunctionType
DR = mybir.MatmulPerfMode.DoubleRow
BF = ml_dtypes.bfloat16
E4 = ml_dtypes.float8_e4m3

B, IN, H, NH = 8192, 1024, 2048, 4
HD = H // NH
EPS = 1e-5
NCORES = 8
BL = B // NCORES          # 1024 batch rows per core
NB = BL // 512            # 2 N-slices of 512
NKH = H // 128            # 16 feature tiles for H-sized dims
NKI = IN // 128           # 8 feature tiles for IN
NJH = NKH // 2            # 8 fp8 pair-groups for H-sized contraction
SCALE = 1.0 / float(np.sqrt(np.float32(HD)))
WS = 64.0                 # fp8 weight pre-scale

_DMA_OPS = ("InstDMACopy", "InstDMATranspose", "InstDMAMemset")


def _to_f32r(a):
    u = np.ascontiguousarray(a, dtype=np.float32).view(np.uint32)
    r = (u + 0x7FF + ((u >> 12) & 1)) & np.uint32(0xFFFFF000)
    return r.view(np.float32)


def _legalize_sync(nc, wait_cap=1, upd_cap=1):
    """This container's walrus supports ~1 sync wait/update per engine
    instruction; hoist the excess onto adjacent NoOps (same engine)."""
    ctr = [0]

    def mknop(eng, waits, upds):
        ctr[0] += 1
        nop = mybir.InstNoOp(name=f"lsync-{ctr[0]}", ins=[], outs=[])
        nop.engine = eng
        nop.sync_info = mybir.SyncInfo(on_wait=list(waits), on_update=list(upds))
        return nop

    for fn in nc.m.functions:
        for blk in fn.blocks:
            out = []
            changed = False
            for ins in blk.instructions:
                si = getattr(ins, "sync_info", None)
                if si is None:
                    out.append(ins)
                    continue
                waits = list(si.on_wait or [])
                upds = list(si.on_update or [])
                pre, post = [], []
                while len(waits) > wait_cap:
                    pre.append(mknop(ins.engine, [waits.pop(0)], []))
                if ins.__class__.__name__ not in _DMA_OPS:
                    while len(upds) > upd_cap:
                        post.append(mknop(ins.engine, [], [upds.pop()]))
                if pre or post:
                    ins.sync_info = mybir.SyncInfo(on_wait=waits, on_update=upds)
                    changed = True
                out.extend(pre)
                out.append(ins)
                out.extend(post)
            if changed:
                try:
                    blk.instructions = out
                except Exception:
                    blk.instructions.clear()
                    blk.instructions.extend(out)


def declare_dram(nc):
    d = {}
    d["h"] = nc.dram_tensor("h", [H, BL], BF16, kind="ExternalInput")
    d["h8"] = nc.dram_tensor("h8", [H, BL], FP8, kind="ExternalInput")
    d["x"] = nc.dram_tensor("x", [IN, BL], BF16, kind="ExternalInput")
    d["wp"] = nc.dram_tensor("wp", [NKH, 128, NKI, 128], BF16, kind="ExternalInput")
    d["wq8"] = nc.dram_tensor("wq8", [NKH, 128, NJH, 2, 128], FP8,
                              kind="ExternalInput")
    d["wg1"] = nc.dram_tensor("wg1", [NKH, 128, NKH, 128], BF16, kind="ExternalInput")
    d["wo"] = nc.dram_tensor("wo", [NKH, 128, NKH, 128], BF16, kind="ExternalInput")
    d["wg2"] = nc.dram_tensor("wg2", [NKH, 128, NKH, 128], BF16, kind="ExternalInput")
    for n in ("bp", "bq", "bo", "bg", "gam", "bet"):
        d[n] = nc.dram_tensor(n, [128, NKH], F32, kind="ExternalInput")
    d["oneseg"] = nc.dram_tensor("oneseg", [128, NH, NH], BF16, kind="ExternalInput")
    d["onescol8"] = nc.dram_tensor("onescol8", [128, 2, 64], FP8,
                                   kind="ExternalInput")
    d["numk"] = nc.dram_tensor("numk", [NH, NH, 128], BF16, kind="ExternalInput")
    d["selg"] = nc.dram_tensor("selg", [NH, NH, 128], BF16, kind="ExternalInput")
    d["onesrow"] = nc.dram_tensor("onesrow", [1, 128], F32R, kind="ExternalInput")
    d["id128"] = nc.dram_tensor("id128", [128, 128], BF16, kind="ExternalInput")
    d["outT"] = nc.dram_tensor("outT", [H, BL], BF16, kind="ExternalOutput")
    return d


def build():
    nc = bass.Bass()
    d = declare_dram(nc)
    with tile.TileContext(nc) as tc:
        _body(nc, tc, d)
    _legalize_sync(nc)
    return nc


def _body(nc, tc, d, stop_after=None):
    def _cut_A():
        nc.sync.dma_start(out=d["outT"][0:128, :], in_=xp[:, 0, :])
        p_h8.release()
        p_xp.release()
        p_g1.release()
        p_out.release()
        p_tmp.release()
        p_ws.release()
        p_h.release()
        consts.release()
        ps_main.release()

    consts = tc.alloc_tile_pool(name="consts", bufs=1, side="left")
    p_h = tc.alloc_tile_pool(name="p_h", bufs=1, side="left")
    p_ws = tc.alloc_tile_pool(name="p_ws", bufs=2, side="left")
    p_tmp = tc.alloc_tile_pool(name="p_tmp", bufs=6, side="left")
    p_out = tc.alloc_tile_pool(name="p_out", bufs=2, side="left")
    p_g1 = tc.alloc_tile_pool(name="p_g1", bufs=1, side="left")
    ps_main = tc.alloc_tile_pool(name="ps_main", bufs=2, space="PSUM", side="left")

    wp_t = {}

    def fetch_wp(m):
        if m < NKH:
            w = p_ws.tile([128, NKI, 128], BF16, tag="wr")
            nc.sync.dma_start(out=w[:, :, :], in_=d["wp"][m, :, :, :])
            wp_t[m] = w

    for m in range(2):
        fetch_wp(m)

    # ---- consts (after first weights in the SP DMA queue) ----
    bt = {}
    for n in ("bp", "bq", "bo", "bg", "gam", "bet"):
        bt[n] = consts.tile([128, NKH], F32, tag='bias_' + n, name='bias_' + n)
        nc.sync.dma_start(out=bt[n], in_=d[n][:, :])
    oneseg = consts.tile([128, NH, NH], BF16)
    nc.sync.dma_start(out=oneseg, in_=d["oneseg"][:, :, :])
    onescol8 = consts.tile([128, 2, 64], FP8)
    nc.sync.dma_start(out=onescol8, in_=d["onescol8"][:, :, :])
    numk = consts.tile([NH, NH, 128], BF16)
    nc.sync.dma_start(out=numk, in_=d["numk"][:, :, :])
    selg = consts.tile([NH, NH, 128], BF16)
    nc.sync.dma_start(out=selg, in_=d["selg"][:, :, :])
    onesrow = consts.tile([1, 128], F32R)
    nc.sync.dma_start(out=onesrow, in_=d["onesrow"][:, :])
    id128 = consts.tile([128, 128], BF16)
    nc.sync.dma_start(out=id128, in_=d["id128"][:, :])
    epst = consts.tile([1, 1], F32)
    nc.vector.memset(epst, EPS)

    # PE p-state warmup during the initial DMA wait: dep-free junk matmuls
    wrm = consts.tile([128, 640], BF16, tag="wrm")
    nc.vector.memset(wrm, 0.0)
    wpo = ps_main.tile([128, 512], F32, tag="po")
    for r in range(14):
        nc.tensor.matmul(wpo[:, :], wrm[:, 0:128], wrm[:, 128:640],
                         start=(r == 0), stop=(r == 13))

    # ---- x first (A needs it immediately) ----
    p_xp = tc.alloc_tile_pool(name="p_xp", bufs=1, side="right")
    p_h8 = tc.alloc_tile_pool(name="p_h8", bufs=1, side="right")
    p_x = tc.alloc_tile_pool(name="p_x", bufs=1, side="right")
    xT = p_x.tile([128, NKI, BL], BF16)
    for k in range(NKI):
        nc.gpsimd.dma_start(out=xT[:, k, :], in_=d["x"][k * 128:(k + 1) * 128, :])

    hT = p_h.tile([128, NKH, BL], BF16)
    h8 = p_h8.tile([128, NKH, BL], FP8)

    # ---- A: xp = Wp.T @ x + bp  (bf16) ----
    xp = p_xp.tile([128, NKH, BL], BF16)
    for m in range(NKH):
        fetch_wp(m + 2)
        if m == 2:      # h8 needed by B; hT by B's products (parallel queue)
            for k in range(NKH):
                nc.gpsimd.dma_start(out=h8[:, k, :],
                                    in_=d["h8"][k * 128:(k + 1) * 128, :])
        if m == 6:
            for k in range(NKH):
                nc.gpsimd.dma_start(out=hT[:, k, :],
                                    in_=d["h"][k * 128:(k + 1) * 128, :])
        w = wp_t.pop(m)
        po = ps_main.tile([128, BL], F32, tag="po")
        for bs in range(NB):
            sl = slice(bs * 512, bs * 512 + 512)
            for k in range(NKI):
                nc.tensor.matmul(po[:, sl], w[:, k, :], xT[:, k, sl],
                                 start=(k == 0), stop=(k == NKI - 1))
        nc.scalar.activation(xp[:, m, :], po[:, :], AF.Identity,
                             bias=bt["bp"][:, m:m + 1], scale=1.0)
    p_x.release()
    if stop_after == "A":
        _cut_A()
        return

    wq_t = {}

    def fetch_wq(m):
        if m < NKH:
            w = p_ws.tile([128, NJH, 2, 128], FP8, tag="wq")
            nc.sync.dma_start(out=w[:, :, :, :], in_=d["wq8"][m, :, :, :, :])
            wq_t[m] = w

    for m in range(2):
        fetch_wq(m)

    # ---- B: q GEMM (fp8 DoubleRow) + products + score reductions ----
    ps_sB = tc.alloc_tile_pool(name="ps_sB", bufs=1, space="PSUM", side="left")
    S = ps_sB.tile([128, BL], F32, tag="sps")   # rows 0-3: s0, 32-35: s1, 64-67: s3
    pend = []
    emit_ctr = [0]

    def emit_scores(g, p0, p1, p3):
        st = emit_ctr[0] == 0
        sp = emit_ctr[0] == NKH - 1
        emit_ctr[0] += 1
        for bs in range(NB):
            sl = slice(bs * 512, bs * 512 + 512)
            nc.tensor.matmul(S[0:4, sl], oneseg[:, g, :], p0[:, sl], start=st, stop=sp)
            nc.tensor.matmul(S[32:36, sl], oneseg[:, g, :], p1[:, sl], start=st, stop=sp)
            nc.tensor.matmul(S[64:68, sl], oneseg[:, g, :], p3[:, sl], start=st, stop=sp)

    for m in range(NKH):
        fetch_wq(m + 2)
        g = m // NH
        w = wq_t.pop(m)
        po = ps_main.tile([128, BL], F32, tag="po")
        for bs in range(NB):
            sl = slice(bs * 512, bs * 512 + 512)
            for j in range(NJH):
                nc.tensor.matmul(po[:, sl], w[:, j, :, :],
                                 h8[:, 2 * j:2 * j + 2, sl],
                                 start=(j == 0), stop=(j == NJH - 1),
                                 perf_mode=DR)
        qm = p_tmp.tile([128, BL], BF16, tag="tb", name="qm")
        nc.scalar.activation(qm[:, :], po[:, :], AF.Identity,
                             bias=bt["bq"][:, m:m + 1], scale=1.0 / WS)
        p0 = p_tmp.tile([128, BL], BF16, tag="tb", name="p0")
        p1 = p_tmp.tile([128, BL], BF16, tag="tb", name="p1")
        p3 = p_tmp.tile([128, BL], BF16, tag="tb", name="p3")
        nc.vector.tensor_mul(p0[:, :], qm[:, :], hT[:, m, :])
        nc.vector.tensor_mul(p1[:, :], qm[:, :], xp[:, m, :])
        nc.vector.tensor_mul(p3[:, :], p0[:, :], xp[:, m, :])
        pend.append((m // NH, p0, p1, p3))
        if len(pend) == 2:
            emit_scores(*pend.pop(0))
    while pend:
        emit_scores(*pend.pop(0))
    p_h8.release()
    if stop_after == "B":
        nc.sync.dma_start(out=d["outT"][0:128, :], in_=xp[:, 0, :])
        ps_sB.release()
        p_xp.release()
        p_g1.release()
        p_out.release()
        p_tmp.release()
        p_ws.release()
        p_h.release()
        consts.release()
        ps_main.release()
        return

    # ---- G1 blocks: g1[m] = (Wg1-Wg2).T @ h + bg, hoisted PE filler ----
    g1 = p_g1.tile([128, NKH, BL], BF16)
    g1_next = [0]
    wg1_t = {}

    def fetch_wg1(m):
        if m < NKH:
            w = p_ws.tile([128, NKH, 128], BF16, tag="wg1")
            nc.sync.dma_start(out=w[:, :, :], in_=d["wg1"][m, :, :, :])
            wg1_t[m] = w

    fetch_wg1(0)

    def g1_block():
        m = g1_next[0]
        if m >= NKH:
            return
        g1_next[0] += 1
        fetch_wg1(m + 1)
        w = wg1_t.pop(m)
        po = ps_main.tile([128, BL], F32, tag="po")
        for bs in range(NB):
            sl = slice(bs * 512, bs * 512 + 512)
            for k in range(NKH):
                nc.tensor.matmul(po[:, sl], w[:, k, :], hT[:, k, sl],
                                 start=(k == 0), stop=(k == NKH - 1))
        nc.scalar.activation(g1[:, m, :], po[:, :], AF.Identity,
                             bias=bt["bg"][:, m:m + 1], scale=1.0)

    g1_block()

    # ---- C: softmax coefficients ----
    p_smA = tc.alloc_tile_pool(name="p_smA", bufs=1, side="right")
    E0 = p_smA.tile([4, BL], BF16)
    E1 = p_smA.tile([4, BL], BF16)
    E2 = p_smA.tile([4, BL], BF16)
    E3 = p_smA.tile([4, BL], BF16)
    nc.scalar.activation(E0[:, :], S[0:4, :], AF.Exp, scale=SCALE)
    nc.scalar.activation(E1[:, :], S[32:36, :], AF.Exp, scale=SCALE)
    nc.scalar.activation(E3[:, :], S[64:68, :], AF.Exp, scale=SCALE)
    nc.vector.tensor_mul(E2[:, :], E0[:, :], E1[:, :])   # exp(s0+s1) == e0*e1
    NUM = ps_sB.tile([128, BL], F32, tag="sps")
    for qi, Eq in enumerate((E0, E1, E2, E3)):
        for bs in range(NB):
            sl = slice(bs * 512, bs * 512 + 512)
            nc.tensor.matmul(NUM[:, sl], numk[:, qi, :], Eq[:, sl],
                             start=(qi == 0), stop=(qi == 3))
    R = p_smA.tile([4, BL], F32)
    nc.vector.reciprocal(R[:, :], NUM[96:100, :])
    Ah = p_smA.tile([4, BL], BF16)
    Ax = p_smA.tile([4, BL], BF16)
    Az = p_smA.tile([4, BL], BF16)
    nc.vector.tensor_mul(Ah[:, :], NUM[0:4, :], R[:, :])
    nc.vector.tensor_mul(Ax[:, :], NUM[32:36, :], R[:, :])
    nc.vector.tensor_mul(Az[:, :], NUM[64:68, :], R[:, :])
    ps_sB.release()

    g1_block()
    g1_block()

    # ---- D1: attn combine (per head, per 512-slice; bf16 coefficients) ----
    p_attn = tc.alloc_tile_pool(name="p_attn", bufs=1, side="left")
    p_cb = tc.alloc_tile_pool(name="p_cb", bufs=2, side="right")
    ps_cD = tc.alloc_tile_pool(name="ps_cD", bufs=1, space="PSUM", side="left")
    # right-stack release order after D1: p_cb, p_smA, p_xp (LIFO)
    attn = p_attn.tile([128, NKH, BL], BF16)
    for g in range(NH):
        for bs in range(NB):
            sl = slice(bs * 512, bs * 512 + 512)
            CH = ps_cD.tile([128, 512], F32, tag="CH")
            CX = ps_cD.tile([128, 512], F32, tag="CX")
            CZ = ps_cD.tile([128, 512], F32, tag="CZ")
            nc.tensor.matmul(CH[:, :], selg[:, g, :], Ah[:, sl], start=True, stop=True)
            nc.tensor.matmul(CX[:, :], selg[:, g, :], Ax[:, sl], start=True, stop=True)
            nc.tensor.matmul(CZ[:, :], selg[:, g, :], Az[:, sl], start=True, stop=True)
            CHb = p_cb.tile([128, 512], BF16, tag="CHb", name="CHb")
            CXb = p_cb.tile([128, 512], BF16, tag="CXb", name="CXb")
            CZb = p_cb.tile([128, 512], BF16, tag="CZb", name="CZb")
            nc.scalar.activation(CHb[:, :], CH[:, :], AF.Identity, scale=1.0)
            nc.scalar.activation(CXb[:, :], CX[:, :], AF.Identity, scale=1.0)
            nc.scalar.activation(CZb[:, :], CZ[:, :], AF.Identity, scale=1.0)
            g1_block()
            for m in range(g * NH, (g + 1) * NH):
                t0 = p_tmp.tile([128, 512], BF16, tag="tf", name="t0", bufs=4)
                t1 = p_tmp.tile([128, 512], BF16, tag="tf", name="t1", bufs=4)
                nc.vector.tensor_mul(t0[:, :], xp[:, m, sl], CZb[:, :])
                nc.vector.tensor_add(t0[:, :], t0[:, :], CHb[:, :])
                nc.vector.tensor_mul(t0[:, :], t0[:, :], hT[:, m, sl])
                nc.vector.tensor_mul(t1[:, :], xp[:, m, sl], CXb[:, :])
                nc.vector.tensor_add(attn[:, m, sl], t0[:, :], t1[:, :])
    ps_cD.release()
    p_cb.release()
    p_smA.release()
    p_xp.release()

    while g1_next[0] < NKH:
        g1_block()
    if stop_after == "D1":
        nc.sync.dma_start(out=d["outT"][0:128, :], in_=attn[:, 0, :])
        p_attn.release()
        p_g1.release()
        p_out.release()
        p_tmp.release()
        p_ws.release()
        p_h.release()
        consts.release()
        ps_main.release()
        return

    # ---- D2 + F fused per m: attn_o = gelu(Wo attn + bo); y = attn_o + h;
    #      LN sums ----
    p_wb = tc.alloc_tile_pool(name="p_wb", bufs=2, side="right")
    p_y = tc.alloc_tile_pool(name="p_y", bufs=1, side="left")
    ps_sF = tc.alloc_tile_pool(name="ps_sF", bufs=1, space="PSUM", side="left")
    y = p_y.tile([128, NKH, BL], BF16)
    SUM = ps_sF.tile([128, BL], F32, tag="sum")

    wo_t = {}
    wg2_t = {}

    def fetch_wo(m):
        if m < NKH:
            w = p_wb.tile([128, NKH, 128], BF16, tag="wb")
            nc.sync.dma_start(out=w[:, :, :], in_=d["wo"][m, :, :, :])
            wo_t[m] = w

    def fetch_wg2(m):
        if m < NKH:
            w = p_wb.tile([128, NKH, 128], BF16, tag="wb")
            nc.sync.dma_start(out=w[:, :, :], in_=d["wg2"][m, :, :, :])
            wg2_t[m] = w

    for m in range(2):
        fetch_wo(m)

    for m in range(NKH):
        fetch_wo(m + 2)
        if m == 14:
            fetch_wg2(0)
        if m == 15:
            fetch_wg2(1)
        w = wo_t.pop(m)
        po = ps_main.tile([128, BL], F32, tag="po")
        for bs in range(NB):
            sl = slice(bs * 512, bs * 512 + 512)
            for k in range(NKH):
                nc.tensor.matmul(po[:, sl], w[:, k, :], attn[:, k, sl],
                                 start=(k == 0), stop=(k == NKH - 1))
        ao_t = p_tmp.tile([128, BL], BF16, tag="tb", name="ao")
        nc.scalar.activation(ao_t[:, :], po[:, :], AF.Gelu,
                             bias=bt["bo"][:, m:m + 1], scale=1.0)
        nc.vector.tensor_add(y[:, m, :], ao_t[:, :], hT[:, m, :])
        yp8 = p_tmp.tile([128, 2, BL], FP8, tag="p8", name="yp8", bufs=3)
        nc.scalar.activation(yp8[:, 0, :], y[:, m, :], AF.Identity, scale=1.0)
        nc.vector.tensor_mul(yp8[:, 1, :], y[:, m, :], y[:, m, :])
        st, sp = (m == 0), (m == NKH - 1)
        for bs in range(NB):
            sl = slice(bs * 512, bs * 512 + 512)
            nc.tensor.matmul(SUM[0:64, sl], onescol8[:, :, :], yp8[:, :, sl],
                             start=st, stop=sp, perf_mode=DR)

    if stop_after == "D2":
        nc.sync.dma_start(out=d["outT"][0:128, :], in_=y[:, 0, :])
        ps_sF.release()
        p_wb.release()
        p_y.release()
        p_attn.release()
        p_g1.release()
        p_out.release()
        p_tmp.release()
        p_ws.release()
        p_h.release()
        consts.release()
        ps_main.release()
        return

    # ---- G: mu / rstd rows + PE broadcast ----
    p_smB = tc.alloc_tile_pool(name="p_smB", bufs=1, side="right")
    MUr = p_smB.tile([1, BL], F32R)
    MSQ = p_smB.tile([1, BL], F32)
    nc.vector.tensor_scalar(out=MUr[:, :], in0=SUM[0:1, :], scalar1=1.0 / H,
                            scalar2=None, op0=mybir.AluOpType.mult)
    nc.vector.tensor_scalar(out=MSQ[:, :], in0=SUM[32:33, :], scalar1=1.0 / H,
                            scalar2=None, op0=mybir.AluOpType.mult)
    MUf = MUr.bitcast(F32)
    MU2 = p_smB.tile([1, BL], F32)
    nc.vector.tensor_mul(MU2[:, :], MUf[:, :], MUf[:, :])
    nc.vector.tensor_sub(MSQ[:, :], MSQ[:, :], MU2[:, :])
    nc.scalar.activation(MU2[:, :], MSQ[:, :], AF.Sqrt, bias=epst[:, 0:1], scale=1.0)
    nc.vector.reciprocal(MSQ[:, :], MU2[:, :])
    RSTr = p_smB.tile([1, BL], F32R)
    nc.vector.tensor_copy(RSTr[:, :], MSQ[:, :])
    ps_sF.release()

    # first two gate-part2 PSUMs before the broadcast matmuls (PE filler)
    def po2_block(m):
        w = wg2_t.pop(m)
        po = ps_main.tile([128, BL], F32, tag="po")
        late = m >= NKH - 2   # inject g1 on the PE for the tail iterations
        for bs in range(NB):
            sl = slice(bs * 512, bs * 512 + 512)
            for k in range(NKH):
                nc.tensor.matmul(po[:, sl], w[:, k, :], y[:, k, sl],
                                 start=(k == 0),
                                 stop=(k == NKH - 1 and not late))
            if late:
                nc.tensor.matmul(po[:, sl], id128[:, :], g1[:, m, sl],
                                 start=False, stop=True)
        return po

    po2_cache = {}
    po2_cache[0] = po2_block(0)

    ps_gh = tc.alloc_tile_pool(name="ps_gh", bufs=1, space="PSUM", side="left")
    MUB = ps_gh.tile([128, BL], F32, tag="mub")
    RSB = ps_gh.tile([128, BL], F32, tag="rsb")
    for bs in range(NB):
        sl = slice(bs * 512, bs * 512 + 512)
        nc.tensor.matmul(MUB[:, sl], onesrow[:, :], MUr[:, sl], start=True, stop=True)
        nc.tensor.matmul(RSB[:, sl], onesrow[:, :], RSTr[:, sl], start=True, stop=True)

    # ---- H: gate sigmoid + normalize + blend + out (fused per m) ----
    for m in range(NKH):
        fetch_wg2(m + 2)
        po = po2_cache.pop(m) if m in po2_cache else po2_block(m)
        gm = p_tmp.tile([128, BL], BF16, tag="tb", name="gm")
        if m >= NKH - 2:
            nc.scalar.activation(gm[:, :], po[:, :], AF.Sigmoid, scale=1.0)
        else:
            pre = p_tmp.tile([128, BL], BF16, tag="tb", name="pre")
            nc.vector.tensor_add(pre[:, :], po[:, :], g1[:, m, :])
            nc.scalar.activation(gm[:, :], pre[:, :], AF.Sigmoid, scale=1.0)
        t0 = p_tmp.tile([128, BL], F32, tag="tf32", name="n0", bufs=2)
        nc.vector.tensor_sub(t0[:, :], y[:, m, :], MUB[:, :])
        nc.vector.tensor_mul(t0[:, :], t0[:, :], RSB[:, :])
        nc.vector.tensor_scalar(out=t0[:, :], in0=t0[:, :],
                                scalar1=bt["gam"][:, m:m + 1],
                                scalar2=bt["bet"][:, m:m + 1],
                                op0=mybir.AluOpType.mult, op1=mybir.AluOpType.add)
        t1 = p_tmp.tile([128, BL], BF16, tag="tb", name="n1")
        blend = nc.vector if m >= NKH - 2 else nc.gpsimd
        blend.tensor_sub(t1[:, :], t0[:, :], hT[:, m, :])
        blend.tensor_mul(t1[:, :], t1[:, :], gm[:, :])
        ot = p_out.tile([128, BL], BF16, tag="ot")
        nc.vector.tensor_add(ot[:, :], t1[:, :], hT[:, m, :])
        nc.sync.dma_start(out=d["outT"][m * 128:(m + 1) * 128, :], in_=ot[:, :])

    p_smB.release()
    p_wb.release()
    p_y.release()
    p_attn.release()
    p_g1.release()
    p_out.release()
    p_tmp.release()
    p_ws.release()
    p_h.release()
    consts.release()
    ps_gh.release()
    ps_main.release()


_NC = None


def _get_nc():
    global _NC
    if _NC is None:
        _NC = build()
    return _NC


def _consts_np():
    oneseg = np.zeros((128, NH, NH), np.float32)
    for g in range(NH):
        oneseg[:, g, g] = 1.0
    numk = np.zeros((NH, NH, 128), np.float32)   # [k=g, q, m]
    for g in range(NH):
        numk[g, 0, g] = 1.0          # e0 -> a_h num
        numk[g, 0, 96 + g] = 1.0     # e0 -> denom
        numk[g, 1, 32 + g] = 1.0     # e1 -> a_xp num
        numk[g, 1, 96 + g] = 1.0
        numk[g, 2, g] = 1.0          # e2 -> a_h num
        numk[g, 2, 32 + g] = 1.0     # e2 -> a_xp num
        numk[g, 2, 96 + g] = 1.0
        numk[g, 3, 64 + g] = 1.0     # e3 -> a_hxp num
        numk[g, 3, 96 + g] = 1.0
    selg = np.zeros((NH, NH, 128), np.float32)   # [k, g, m]
    for g in range(NH):
        selg[g, g, :] = 1.0
    onescol8 = np.zeros((128, 2, 64), np.float32)
    onescol8[:, 0, 0] = 1.0    # slot0 (y)   -> SUM row 0
    onescol8[:, 1, 32] = 1.0   # slot1 (y^2) -> SUM row 32
    return dict(
        oneseg=oneseg.astype(BF),
        onescol8=onescol8.astype(E4),
        numk=numk.astype(BF),
        selg=selg.astype(BF),
        onesrow=_to_f32r(np.ones((1, 128), np.float32)),
        id128=np.eye(128, dtype=np.float32).astype(BF),
    )


def _vec16(v):
    return np.ascontiguousarray(np.asarray(v, np.float32).reshape(NKH, 128).T)


def prepare_in_maps(h_prev, x, W_proj, b_proj, W_q, b_q, W_o, b_o, W_g, b_g,
                    gamma, beta):
    def _pack(wT):
        # [K, M] -> [m, p, k, c] contiguous (per-partition 8KB chunks)
        K_, M_ = wT.shape
        return np.ascontiguousarray(
            wT.reshape(K_ // 128, 128, M_ // 128, 128).transpose(2, 1, 0, 3))

    def _pack8(wT):
        # [K, M] -> [m, p, j, i, c]: fp8 pair-groups of 256 contraction rows
        K_, M_ = wT.shape
        w8 = (np.asarray(wT, np.float32) * WS).astype(E4)
        return np.ascontiguousarray(
            w8.reshape(K_ // 256, 2, 128, M_ // 128, 128).transpose(3, 2, 0, 1, 4))

    Wg = np.asarray(W_g, np.float32)
    shared = {
        "wp": _pack(np.asarray(W_proj, np.float32).T).astype(BF),
        "wq8": _pack8(np.asarray(W_q, np.float32).T),
        "wg1": _pack((Wg[:, :H] - Wg[:, H:]).T).astype(BF),
        "wo": _pack(np.asarray(W_o, np.float32).T).astype(BF),
        "wg2": _pack(Wg[:, H:].T).astype(BF),
        "bp": _vec16(b_proj), "bq": _vec16(b_q), "bo": _vec16(b_o),
        "bg": _vec16(b_g), "gam": _vec16(gamma), "bet": _vec16(beta),
    }
    shared.update(_consts_np())
    h2 = np.asarray(h_prev, np.float32).reshape(B, H)
    x2 = np.asarray(x, np.float32)
    in_maps = []
    for c in range(NCORES):
        m = dict(shared)
        hc = np.ascontiguousarray(h2[c * BL:(c + 1) * BL].T)
        m["h"] = hc.astype(BF)
        m["h8"] = hc.astype(E4)
        m["x"] = np.ascontiguousarray(x2[c * BL:(c + 1) * BL].T).astype(BF)
        in_maps.append(m)
    return in_maps


def run_device(in_maps, **kw):
    nc = _get_nc()
    return run_bass_kernel_spmd(nc, in_maps, core_ids=list(range(NCORES)), **kw)


_RUNNER = None


def _get_runner():
    """Custom sharded runner: per-core tensors sharded on the core axis,
    replicated weights/consts transferred once (not 8x)."""
    global _RUNNER
    if _RUNNER is not None:
        return _RUNNER
    import jax
    from jax.sharding import Mesh, PartitionSpec, NamedSharding
    try:
        from jax import shard_map as _sm
        shard_map = _sm.shard_map if hasattr(_sm, "shard_map") else _sm
    except Exception:
        from jax.experimental.shard_map import shard_map
    from concourse.bass2jax import _bass_exec_p, partition_id_tensor, \
        install_neuronx_cc_hook
    install_neuronx_cc_hook()

    nc = _get_nc()
    pid_name = nc.partition_id_tensor.name if nc.partition_id_tensor else None
    in_names, out_names, out_avals = [], [], []
    for alloc in nc.m.functions[0].allocations:
        if not isinstance(alloc, mybir.MemoryLocationSet):
            continue
        name = alloc.memorylocations[0].name
        if alloc.kind == "ExternalInput" and name != pid_name:
            in_names.append(name)
        elif alloc.kind == "ExternalOutput":
            out_names.append(name)
            out_avals.append(jax.core.ShapedArray(
                tuple(alloc.tensor_shape), mybir.dt.np(alloc.dtype)))
    bind_names = in_names + out_names + ([pid_name] if pid_name else [])
    sharded_names = {"h", "h8", "x"}

    def _body_fn(*args):
        operands = list(args)
        operands.append(partition_id_tensor())
        return tuple(_bass_exec_p.bind(
            *operands,
            out_avals=tuple(out_avals),
            in_names=tuple(bind_names),
            out_names=tuple(out_names),
            lowering_input_output_aliases=(),
            sim_require_finite=True,
            sim_require_nnan=True,
            nc=nc,
        ))

    devices = jax.devices()[:NCORES]
    mesh = Mesh(np.asarray(devices), ("core",))
    Pc, Pr = PartitionSpec("core"), PartitionSpec()
    in_specs = tuple(Pc if n in sharded_names else Pr for n in in_names) \
        + (Pc,) * len(out_names)
    import inspect
    _smkw = {}
    try:
        _p = inspect.signature(shard_map).parameters
        _smkw["check_rep" if "check_rep" in _p else "check_vma"] = False
    except Exception:
        _smkw["check_rep"] = False
    fn = jax.jit(
        shard_map(_body_fn, mesh=mesh, in_specs=in_specs,
                  out_specs=(Pc,) * len(out_names), **_smkw),
        keep_unused=True)
    dev_zeros = [
        jax.device_put(
            np.zeros((NCORES * av.shape[0], *av.shape[1:]), av.dtype),
            NamedSharding(mesh, Pc))
        for av in out_avals
    ]
    _RUNNER = (fn, mesh, in_names, out_names, out_avals, sharded_names, dev_zeros)
    return _RUNNER


def run_device_fast(in_maps):
    fn, mesh, in_names, out_names, out_avals, sharded_names, dev_zeros = _get_runner()
    args = []
    for n in in_names:
        if n in sharded_names:
            args.append(np.concatenate([np.asarray(m[n]) for m in in_maps], axis=0))
        else:
            args.append(np.asarray(in_maps[0][n]))
    args.extend(dev_zeros)
    outs = fn(*args)
    return {name: np.asarray(outs[i]) for i, name in enumerate(out_names)}


def kernel(**inputs):
    in_maps = prepare_in_maps(**inputs)
    try:
        outs = run_device_fast(in_maps)
        big = outs["outT"].reshape(NCORES, H, BL)
        out = np.empty((B, H), np.float32)
        for c in range(NCORES):
            out[c * BL:(c + 1) * BL] = big[c].T.astype(np.float32)
    except Exception:
        res = run_device(in_maps)
        out = np.empty((B, H), np.float32)
        for c in range(NCORES):
            out[c * BL:(c + 1) * BL] = np.asarray(
                res.results[c]["outT"], np.float32).T
    return out.reshape(B, 1, H)


# revision 21
# speedup vs baseline: 1.0803x; 1.0803x over previous
"""AttnRNNCell Trainium2 kernel: 8-core data-parallel over batch.

Feature-major activations [H, B_local]; bf16 matmuls except the q GEMM,
which runs in fp8e4 DoubleRow (2x PE rate; softmax damps the quantization).
The h-only half of the gate GEMM (wg1 = Wg1-Wg2 acting on h) is hoisted into
the softmax/attn-combine window to keep the PE busy, accumulated to SBUF, and
re-injected into the gate PSUM via an identity-stationary matmul.

Gate algebra: sigmoid(Wg1 h + Wg2 attn_o + bg) == sigmoid((Wg1-Wg2) h +
Wg2 (attn_o + h) + bg), so the gate GEMM part2 consumes y = attn_o + h.
"""
import sys

sys.path.insert(0, "/opt/trn_rl_repo")

import numpy as np
import ml_dtypes

import concourse.bass as bass
import concourse.tile as tile
import concourse.mybir as mybir
from concourse.bass_utils import run_bass_kernel_spmd

F32 = mybir.dt.float32
F32R = mybir.dt.float32r
BF16 = mybir.dt.bfloat16
FP8 = mybir.dt.float8e4
AF = mybir.ActivationFunctionType
DR = mybir.MatmulPerfMode.DoubleRow
BF = ml_dtypes.bfloat16
E4 = ml_dtypes.float8_e4m3

B, IN, H, NH = 8192, 1024, 2048, 4
HD = H // NH
EPS = 1e-5
NCORES = 8
BL = B // NCORES          # 1024 batch rows per core
NB = BL // 512            # 2 N-slices of 512
NKH = H // 128            # 16 feature tiles for H-sized dims
NKI = IN // 128           # 8 feature tiles for IN
NJH = NKH // 2            # 8 fp8 pair-groups for H-sized contraction
SCALE = 1.0 / float(np.sqrt(np.float32(HD)))
WS = 64.0                 # fp8 weight pre-scale

_DMA_OPS = ("InstDMACopy", "InstDMATranspose", "InstDMAMemset")


def _to_f32r(a):
    u = np.ascontiguousarray(a, dtype=np.float32).view(np.uint32)
    r = (u + 0x7FF + ((u >> 12) & 1)) & np.uint32(0xFFFFF000)
    return r.view(np.float32)


def _legalize_sync(nc, wait_cap=1, upd_cap=1):
    """This container's walrus supports ~1 sync wait/update per engine
    instruction; hoist the excess onto adjacent NoOps (same engine)."""
    ctr = [0]

    def mknop(eng, waits, upds):
        ctr[0] += 1
        nop = mybir.InstNoOp(name=f"lsync-{ctr[0]}", ins=[], outs=[])
        nop.engine = eng
        nop.sync_info = mybir.SyncInfo(on_wait=list(waits), on_update=list(upds))
        return nop

    for fn in nc.m.functions:
        for blk in fn.blocks:
            out = []
            changed = False
            for ins in blk.instructions:
                si = getattr(ins, "sync_info", None)
                if si is None:
                    out.append(ins)
                    continue
                waits = list(si.on_wait or [])
                upds = list(si.on_update or [])
                pre, post = [], []
                while len(waits) > wait_cap:
                    pre.append(mknop(ins.engine, [waits.pop(0)], []))
                if ins.__class__.__name__ not in _DMA_OPS:
                    while len(upds) > upd_cap:
                        post.append(mknop(ins.engine, [], [upds.pop()]))
                if pre or post:
                    ins.sync_info = mybir.SyncInfo(on_wait=waits, on_update=upds)
                    changed = True
                out.extend(pre)
                out.append(ins)
                out.extend(post)
            if changed:
                try:
                    blk.instructions = out
                except Exception:
                    blk.instructions.clear()
                    blk.instructions.extend(out)


def declare_dram(nc):
    d = {}
    d["h"] = nc.dram_tensor("h", [H, BL], BF16, kind="ExternalInput")
    d["h8"] = nc.dram_tensor("h8", [H, BL], FP8, kind="ExternalInput")
    d["x"] = nc.dram_tensor("x", [IN, BL], BF16, kind="ExternalInput")
    d["wp"] = nc.dram_tensor("wp", [NKH, 128, NKI, 128], BF16, kind="ExternalInput")
    d["wq8"] = nc.dram_tensor("wq8", [NKH, 128, NJH, 2, 128], FP8,
                              kind="ExternalInput")
    d["wg1"] = nc.dram_tensor("wg1", [NKH, 128, NKH, 128], BF16, kind="ExternalInput")
    d["wo"] = nc.dram_tensor("wo", [NKH, 128, NKH, 128], BF16, kind="ExternalInput")
    d["wg2"] = nc.dram_tensor("wg2", [NKH, 128, NKH, 128], BF16, kind="ExternalInput")
    for n in ("bp", "bq", "bo", "bg", "gam", "bet"):
        d[n] = nc.dram_tensor(n, [128, NKH], F32, kind="ExternalInput")
    d["oneseg"] = nc.dram_tensor("oneseg", [128, NH, NH], BF16, kind="ExternalInput")
    d["onescol8"] = nc.dram_tensor("onescol8", [128, 2, 64], FP8,
                                   kind="ExternalInput")
    d["numk"] = nc.dram_tensor("numk", [NH, NH, 128], BF16, kind="ExternalInput")
    d["selg"] = nc.dram_tensor("selg", [NH, NH, 128], BF16, kind="ExternalInput")
    d["onesrow"] = nc.dram_tensor("onesrow", [1, 128], F32R, kind="ExternalInput")
    d["id128"] = nc.dram_tensor("id128", [128, 128], BF16, kind="ExternalInput")
    d["outT"] = nc.dram_tensor("outT", [H, BL], BF16, kind="ExternalOutput")
    return d


def build():
    nc = bass.Bass()
    d = declare_dram(nc)
    with tile.TileContext(nc) as tc:
        _body(nc, tc, d)
    _legalize_sync(nc)
    return nc


def _body(nc, tc, d, stop_after=None):
    def _cut_A():
        nc.sync.dma_start(out=d["outT"][0:128, :], in_=xp[:, 0, :])
        p_h8.release()
        p_xp.release()
        p_g1.release()
        p_out.release()
        p_tmp.release()
        p_ws.release()
        p_h.release()
        consts.release()
        ps_main.release()

    consts = tc.alloc_tile_pool(name="consts", bufs=1, side="left")
    p_h = tc.alloc_tile_pool(name="p_h", bufs=1, side="left")
    p_ws = tc.alloc_tile_pool(name="p_ws", bufs=2, side="left")
    p_tmp = tc.alloc_tile_pool(name="p_tmp", bufs=6, side="left")
    p_out = tc.alloc_tile_pool(name="p_out", bufs=2, side="left")
    p_g1 = tc.alloc_tile_pool(name="p_g1", bufs=1, side="left")
    ps_main = tc.alloc_tile_pool(name="ps_main", bufs=2, space="PSUM", side="left")

    wp_t = {}

    def fetch_wp(m):
        if m < NKH:
            w = p_ws.tile([128, NKI, 128], BF16, tag="wr")
            nc.sync.dma_start(out=w[:, :, :], in_=d["wp"][m, :, :, :])
            wp_t[m] = w

    for m in range(2):
        fetch_wp(m)

    # ---- consts (after first weights in the SP DMA queue) ----
    bt = {}
    for n in ("bp", "bq", "bo", "bg", "gam", "bet"):
        bt[n] = consts.tile([128, NKH], F32, tag='bias_' + n, name='bias_' + n)
        nc.sync.dma_start(out=bt[n], in_=d[n][:, :])
    oneseg = consts.tile([128, NH, NH], BF16)
    nc.sync.dma_start(out=oneseg, in_=d["oneseg"][:, :, :])
    onescol8 = consts.tile([128, 2, 64], FP8)
    nc.sync.dma_start(out=onescol8, in_=d["onescol8"][:, :, :])
    numk = consts.tile([NH, NH, 128], BF16)
    nc.sync.dma_start(out=numk, in_=d["numk"][:, :, :])
    selg = consts.tile([NH, NH, 128], BF16)
    nc.sync.dma_start(out=selg, in_=d["selg"][:, :, :])
    onesrow = consts.tile([1, 128], F32R)
    nc.sync.dma_start(out=onesrow, in_=d["onesrow"][:, :])
    id128 = consts.tile([128, 128], BF16)
    nc.sync.dma_start(out=id128, in_=d["id128"][:, :])
    epst = consts.tile([1, 1], F32)
    nc.vector.memset(epst, EPS)

    # PE p-state warmup during the initial DMA wait: dep-free junk matmuls
    wrm = consts.tile([128, 640], BF16, tag="wrm")
    nc.vector.memset(wrm, 0.0)
    wpo = ps_main.tile([128, 512], F32, tag="po")
    for r in range(6):
        nc.tensor.matmul(wpo[:, :], wrm[:, 0:128], wrm[:, 128:640],
                         start=(r == 0), stop=(r == 5))

    # ---- x first (A needs it immediately) ----
    p_xp = tc.alloc_tile_pool(name="p_xp", bufs=1, side="right")
    p_h8 = tc.alloc_tile_pool(name="p_h8", bufs=1, side="right")
    p_x = tc.alloc_tile_pool(name="p_x", bufs=1, side="right")
    xT = p_x.tile([128, NKI, BL], BF16)
    for k in range(NKI):
        nc.gpsimd.dma_start(out=xT[:, k, :], in_=d["x"][k * 128:(k + 1) * 128, :])

    hT = p_h.tile([128, NKH, BL], BF16)
    h8 = p_h8.tile([128, NKH, BL], FP8)

    # ---- A: xp = Wp.T @ x + bp  (bf16) ----
    xp = p_xp.tile([128, NKH, BL], BF16)
    for m in range(NKH):
        fetch_wp(m + 2)
        if m == 2:      # h8 needed by B; hT by B's products (parallel queue)
            for k in range(NKH):
                nc.gpsimd.dma_start(out=h8[:, k, :],
                                    in_=d["h8"][k * 128:(k + 1) * 128, :])
        if m == 6:
            for k in range(NKH):
                nc.gpsimd.dma_start(out=hT[:, k, :],
                                    in_=d["h"][k * 128:(k + 1) * 128, :])
        w = wp_t.pop(m)
        po = ps_main.tile([128, BL], F32, tag="po")
        for bs in range(NB):
            sl = slice(bs * 512, bs * 512 + 512)
            for k in range(NKI):
                nc.tensor.matmul(po[:, sl], w[:, k, :], xT[:, k, sl],
                                 start=(k == 0), stop=(k == NKI - 1))
        nc.scalar.activation(xp[:, m, :], po[:, :], AF.Identity,
                             bias=bt["bp"][:, m:m + 1], scale=1.0)
    p_x.release()
    if stop_after == "A":
        _cut_A()
        return

    wq_t = {}

    def fetch_wq(m):
        if m < NKH:
            w = p_ws.tile([128, NJH, 2, 128], FP8, tag="wq")
            nc.sync.dma_start(out=w[:, :, :, :], in_=d["wq8"][m, :, :, :, :])
            wq_t[m] = w

    for m in range(2):
        fetch_wq(m)

    # ---- B: q GEMM (fp8 DoubleRow) + products + score reductions ----
    ps_sB = tc.alloc_tile_pool(name="ps_sB", bufs=1, space="PSUM", side="left")
    S = ps_sB.tile([128, BL], F32, tag="sps")   # rows 0-3: s0, 32-35: s1, 64-67: s3
    pend = []
    emit_ctr = [0]

    def emit_scores(g, p0, p1, p3):
        st = emit_ctr[0] == 0
        sp = emit_ctr[0] == NKH - 1
        emit_ctr[0] += 1
        for bs in range(NB):
            sl = slice(bs * 512, bs * 512 + 512)
            nc.tensor.matmul(S[0:4, sl], oneseg[:, g, :], p0[:, sl], start=st, stop=sp)
            nc.tensor.matmul(S[32:36, sl], oneseg[:, g, :], p1[:, sl], start=st, stop=sp)
            nc.tensor.matmul(S[64:68, sl], oneseg[:, g, :], p3[:, sl], start=st, stop=sp)

    for m in range(NKH):
        fetch_wq(m + 2)
        g = m // NH
        w = wq_t.pop(m)
        po = ps_main.tile([128, BL], F32, tag="po")
        for bs in range(NB):
            sl = slice(bs * 512, bs * 512 + 512)
            for j in range(NJH):
                nc.tensor.matmul(po[:, sl], w[:, j, :, :],
                                 h8[:, 2 * j:2 * j + 2, sl],
                                 start=(j == 0), stop=(j == NJH - 1),
                                 perf_mode=DR)
        qm = p_tmp.tile([128, BL], BF16, tag="tb", name="qm")
        nc.scalar.activation(qm[:, :], po[:, :], AF.Identity,
                             bias=bt["bq"][:, m:m + 1], scale=1.0 / WS)
        p0 = p_tmp.tile([128, BL], BF16, tag="tb", name="p0")
        p1 = p_tmp.tile([128, BL], BF16, tag="tb", name="p1")
        p3 = p_tmp.tile([128, BL], BF16, tag="tb", name="p3")
        nc.vector.tensor_mul(p0[:, :], qm[:, :], hT[:, m, :])
        nc.vector.tensor_mul(p1[:, :], qm[:, :], xp[:, m, :])
        nc.vector.tensor_mul(p3[:, :], p0[:, :], xp[:, m, :])
        pend.append((m // NH, p0, p1, p3))
        if len(pend) == 2:
            emit_scores(*pend.pop(0))
    while pend:
        emit_scores(*pend.pop(0))
    p_h8.release()
    if stop_after == "B":
        nc.sync.dma_start(out=d["outT"][0:128, :], in_=xp[:, 0, :])
        ps_sB.release()
        p_xp.release()
        p_g1.release()
        p_out.release()
        p_tmp.release()
        p_ws.release()
        p_h.release()
        consts.release()
        ps_main.release()
        return

    # ---- G1 blocks: g1[m] = (Wg1-Wg2).T @ h + bg, hoisted PE filler ----
    g1 = p_g1.tile([128, NKH, BL], BF16)
    g1_next = [0]
    wg1_t = {}

    def fetch_wg1(m):
        if m < NKH:
            w = p_ws.tile([128, NKH, 128], BF16, tag="wg1")
            nc.sync.dma_start(out=w[:, :, :], in_=d["wg1"][m, :, :, :])
            wg1_t[m] = w

    fetch_wg1(0)

    def g1_block():
        m = g1_next[0]
        if m >= NKH:
            return
        g1_next[0] += 1
        fetch_wg1(m + 1)
        w = wg1_t.pop(m)
        po = ps_main.tile([128, BL], F32, tag="po")
        for bs in range(NB):
            sl = slice(bs * 512, bs * 512 + 512)
            for k in range(NKH):
                nc.tensor.matmul(po[:, sl], w[:, k, :], hT[:, k, sl],
                                 start=(k == 0), stop=(k == NKH - 1))
        nc.scalar.activation(g1[:, m, :], po[:, :], AF.Identity,
                             bias=bt["bg"][:, m:m + 1], scale=1.0)

    g1_block()

    # ---- C: softmax coefficients ----
    p_smA = tc.alloc_tile_pool(name="p_smA", bufs=1, side="right")
    E0 = p_smA.tile([4, BL], BF16)
    E1 = p_smA.tile([4, BL], BF16)
    E2 = p_smA.tile([4, BL], BF16)
    E3 = p_smA.tile([4, BL], BF16)
    nc.scalar.activation(E0[:, :], S[0:4, :], AF.Exp, scale=SCALE)
    nc.scalar.activation(E1[:, :], S[32:36, :], AF.Exp, scale=SCALE)
    nc.scalar.activation(E3[:, :], S[64:68, :], AF.Exp, scale=SCALE)
    nc.vector.tensor_mul(E2[:, :], E0[:, :], E1[:, :])   # exp(s0+s1) == e0*e1
    NUM = ps_sB.tile([128, BL], F32, tag="sps")
    for qi, Eq in enumerate((E0, E1, E2, E3)):
        for bs in range(NB):
            sl = slice(bs * 512, bs * 512 + 512)
            nc.tensor.matmul(NUM[:, sl], numk[:, qi, :], Eq[:, sl],
                             start=(qi == 0), stop=(qi == 3))
    R = p_smA.tile([4, BL], F32)
    nc.vector.reciprocal(R[:, :], NUM[96:100, :])
    Ah = p_smA.tile([4, BL], BF16)
    Ax = p_smA.tile([4, BL], BF16)
    Az = p_smA.tile([4, BL], BF16)
    nc.vector.tensor_mul(Ah[:, :], NUM[0:4, :], R[:, :])
    nc.vector.tensor_mul(Ax[:, :], NUM[32:36, :], R[:, :])
    nc.vector.tensor_mul(Az[:, :], NUM[64:68, :], R[:, :])
    ps_sB.release()

    g1_block()
    g1_block()

    # ---- D1: attn combine (per head, per 512-slice; bf16 coefficients) ----
    p_attn = tc.alloc_tile_pool(name="p_attn", bufs=1, side="left")
    p_cb = tc.alloc_tile_pool(name="p_cb", bufs=2, side="right")
    ps_cD = tc.alloc_tile_pool(name="ps_cD", bufs=1, space="PSUM", side="left")
    # right-stack release order after D1: p_cb, p_smA, p_xp (LIFO)
    attn = p_attn.tile([128, NKH, BL], BF16)
    for g in range(NH):
        for bs in range(NB):
            sl = slice(bs * 512, bs * 512 + 512)
            CH = ps_cD.tile([128, 512], F32, tag="CH")
            CX = ps_cD.tile([128, 512], F32, tag="CX")
            CZ = ps_cD.tile([128, 512], F32, tag="CZ")
            nc.tensor.matmul(CH[:, :], selg[:, g, :], Ah[:, sl], start=True, stop=True)
            nc.tensor.matmul(CX[:, :], selg[:, g, :], Ax[:, sl], start=True, stop=True)
            nc.tensor.matmul(CZ[:, :], selg[:, g, :], Az[:, sl], start=True, stop=True)
            CHb = p_cb.tile([128, 512], BF16, tag="CHb", name="CHb")
            CXb = p_cb.tile([128, 512], BF16, tag="CXb", name="CXb")
            CZb = p_cb.tile([128, 512], BF16, tag="CZb", name="CZb")
            nc.scalar.activation(CHb[:, :], CH[:, :], AF.Identity, scale=1.0)
            nc.scalar.activation(CXb[:, :], CX[:, :], AF.Identity, scale=1.0)
            nc.scalar.activation(CZb[:, :], CZ[:, :], AF.Identity, scale=1.0)
            g1_block()
            for m in range(g * NH, (g + 1) * NH):
                t0 = p_tmp.tile([128, 512], BF16, tag="tf", name="t0", bufs=4)
                t1 = p_tmp.tile([128, 512], BF16, tag="tf", name="t1", bufs=4)
                nc.vector.tensor_mul(t0[:, :], xp[:, m, sl], CZb[:, :])
                nc.vector.tensor_add(t0[:, :], t0[:, :], CHb[:, :])
                nc.vector.tensor_mul(t0[:, :], t0[:, :], hT[:, m, sl])
                nc.vector.tensor_mul(t1[:, :], xp[:, m, sl], CXb[:, :])
                nc.vector.tensor_add(attn[:, m, sl], t0[:, :], t1[:, :])
    ps_cD.release()
    p_cb.release()
    p_smA.release()
    p_xp.release()

    while g1_next[0] < NKH:
        g1_block()
    if stop_after == "D1":
        nc.sync.dma_start(out=d["outT"][0:128, :], in_=attn[:, 0, :])
        p_attn.release()
        p_g1.release()
        p_out.release()
        p_tmp.release()
        p_ws.release()
        p_h.release()
        consts.release()
        ps_main.release()
        return

    # ---- D2 + F fused per m: attn_o = gelu(Wo attn + bo); y = attn_o + h;
    #      LN sums ----
    p_wb = tc.alloc_tile_pool(name="p_wb", bufs=3, side="right")
    p_y = tc.alloc_tile_pool(name="p_y", bufs=1, side="left")
    ps_sF = tc.alloc_tile_pool(name="ps_sF", bufs=1, space="PSUM", side="left")
    y = p_y.tile([128, NKH, BL], BF16)
    SUM = ps_sF.tile([128, BL], F32, tag="sum")

    wo_t = {}
    wg2_t = {}

    def fetch_wo(m):
        if m < NKH:
            w = p_wb.tile([128, NKH, 128], BF16, tag="wb")
            nc.sync.dma_start(out=w[:, :, :], in_=d["wo"][m, :, :, :])
            wo_t[m] = w

    def fetch_wg2(m):
        if m < NKH:
            w = p_wb.tile([128, NKH, 128], BF16, tag="wb")
            nc.sync.dma_start(out=w[:, :, :], in_=d["wg2"][m, :, :, :])
            wg2_t[m] = w

    for m in range(2):
        fetch_wo(m)

    for m in range(NKH):
        fetch_wo(m + 2)
        if m == 14:
            fetch_wg2(0)
        if m == 15:
            fetch_wg2(1)
        w = wo_t.pop(m)
        po = ps_main.tile([128, BL], F32, tag="po")
        for bs in range(NB):
            sl = slice(bs * 512, bs * 512 + 512)
            for k in range(NKH):
                nc.tensor.matmul(po[:, sl], w[:, k, :], attn[:, k, sl],
                                 start=(k == 0), stop=(k == NKH - 1))
        ao_t = p_tmp.tile([128, BL], BF16, tag="tb", name="ao")
        nc.scalar.activation(ao_t[:, :], po[:, :], AF.Gelu,
                             bias=bt["bo"][:, m:m + 1], scale=1.0)
        nc.vector.tensor_add(y[:, m, :], ao_t[:, :], hT[:, m, :])
        yp8 = p_tmp.tile([128, 2, BL], FP8, tag="p8", name="yp8", bufs=3)
        nc.scalar.activation(yp8[:, 0, :], y[:, m, :], AF.Identity, scale=1.0)
        nc.vector.tensor_mul(yp8[:, 1, :], y[:, m, :], y[:, m, :])
        st, sp = (m == 0), (m == NKH - 1)
        for bs in range(NB):
            sl = slice(bs * 512, bs * 512 + 512)
            nc.tensor.matmul(SUM[0:64, sl], onescol8[:, :, :], yp8[:, :, sl],
                             start=st, stop=sp, perf_mode=DR)

    if stop_after == "D2":
        nc.sync.dma_start(out=d["outT"][0:128, :], in_=y[:, 0, :])
        ps_sF.release()
        p_wb.release()
        p_y.release()
        p_attn.release()
        p_g1.release()
        p_out.release()
        p_tmp.release()
        p_ws.release()
        p_h.release()
        consts.release()
        ps_main.release()
        return

    # ---- G: mu / rstd rows + PE broadcast ----
    p_smB = tc.alloc_tile_pool(name="p_smB", bufs=1, side="right")
    MUr = p_smB.tile([1, BL], F32R)
    MSQ = p_smB.tile([1, BL], F32)
    nc.vector.tensor_scalar(out=MUr[:, :], in0=SUM[0:1, :], scalar1=1.0 / H,
                            scalar2=None, op0=mybir.AluOpType.mult)
    nc.vector.tensor_scalar(out=MSQ[:, :], in0=SUM[32:33, :], scalar1=1.0 / H,
                            scalar2=None, op0=mybir.AluOpType.mult)
    MUf = MUr.bitcast(F32)
    MU2t = p_tmp.tile([128, BL], F32, tag="tf32", name="mu2", bufs=2)
    MU2 = MU2t[0:1, :]
    nc.vector.tensor_mul(MU2, MUf[:, :], MUf[:, :])
    nc.vector.tensor_sub(MSQ[:, :], MSQ[:, :], MU2)
    nc.scalar.activation(MU2, MSQ[:, :], AF.Sqrt, bias=epst[:, 0:1], scale=1.0)
    nc.vector.reciprocal(MSQ[:, :], MU2)
    RSTr = p_smB.tile([1, BL], F32R)
    nc.vector.tensor_copy(RSTr[:, :], MSQ[:, :])
    ps_sF.release()

    # first two gate-part2 PSUMs before the broadcast matmuls (PE filler)
    def po2_block(m):
        w = wg2_t.pop(m)
        po = ps_main.tile([128, BL], F32, tag="po")
        late = m >= NKH - 2   # inject g1 on the PE for the tail iterations
        for bs in range(NB):
            sl = slice(bs * 512, bs * 512 + 512)
            for k in range(NKH):
                nc.tensor.matmul(po[:, sl], w[:, k, :], y[:, k, sl],
                                 start=(k == 0),
                                 stop=(k == NKH - 1 and not late))
            if late:
                nc.tensor.matmul(po[:, sl], id128[:, :], g1[:, m, sl],
                                 start=False, stop=True)
        return po

    po2_cache = {}
    po2_cache[0] = po2_block(0)

    ps_gh = tc.alloc_tile_pool(name="ps_gh", bufs=1, space="PSUM", side="left")
    MUB = ps_gh.tile([128, BL], F32, tag="mub")
    RSB = ps_gh.tile([128, BL], F32, tag="rsb")
    for bs in range(NB):
        sl = slice(bs * 512, bs * 512 + 512)
        nc.tensor.matmul(MUB[:, sl], onesrow[:, :], MUr[:, sl], start=True, stop=True)
        nc.tensor.matmul(RSB[:, sl], onesrow[:, :], RSTr[:, sl], start=True, stop=True)

    # ---- H: gate sigmoid + normalize + blend + out (fused per m) ----
    for m in range(NKH):
        fetch_wg2(m + 2)
        po = po2_cache.pop(m) if m in po2_cache else po2_block(m)
        gm = p_tmp.tile([128, BL], BF16, tag="tb", name="gm")
        if m >= NKH - 2:
            nc.scalar.activation(gm[:, :], po[:, :], AF.Sigmoid, scale=1.0)
        else:
            pre = p_tmp.tile([128, BL], BF16, tag="tb", name="pre")
            nc.vector.tensor_add(pre[:, :], po[:, :], g1[:, m, :])
            nc.scalar.activation(gm[:, :], pre[:, :], AF.Sigmoid, scale=1.0)
        t0 = p_tmp.tile([128, BL], F32, tag="tf32", name="n0", bufs=2)
        nc.vector.tensor_sub(t0[:, :], y[:, m, :], MUB[:, :])
        nc.vector.tensor_mul(t0[:, :], t0[:, :], RSB[:, :])
        nc.vector.tensor_scalar(out=t0[:, :], in0=t0[:, :],
                                scalar1=bt["gam"][:, m:m + 1],
                                scalar2=bt["bet"][:, m:m + 1],
                                op0=mybir.AluOpType.mult, op1=mybir.AluOpType.add)
        t1 = p_tmp.tile([128, BL], BF16, tag="tb", name="n1")
        blend = nc.vector if m >= NKH - 2 else nc.gpsimd
        blend.tensor_sub(t1[:, :], t0[:, :], hT[:, m, :])
        blend.tensor_mul(t1[:, :], t1[:, :], gm[:, :])
        ot = p_out.tile([128, BL], BF16, tag="ot")
        nc.vector.tensor_add(ot[:, :], t1[:, :], hT[:, m, :])
        nc.sync.dma_start(out=d["outT"][m * 128:(m + 1) * 128, :], in_=ot[:, :])

    p_smB.release()
    p_wb.release()
    p_y.release()
    p_attn.release()
    p_g1.release()
    p_out.release()
    p_tmp.release()
    p_ws.release()
    p_h.release()
    consts.release()
    ps_gh.release()
    ps_main.release()


_NC = None


def _get_nc():
    global _NC
    if _NC is None:
        _NC = build()
    return _NC


def _consts_np():
    oneseg = np.zeros((128, NH, NH), np.float32)
    for g in range(NH):
        oneseg[:, g, g] = 1.0
    numk = np.zeros((NH, NH, 128), np.float32)   # [k=g, q, m]
    for g in range(NH):
        numk[g, 0, g] = 1.0          # e0 -> a_h num
        numk[g, 0, 96 + g] = 1.0     # e0 -> denom
        numk[g, 1, 32 + g] = 1.0     # e1 -> a_xp num
        numk[g, 1, 96 + g] = 1.0
        numk[g, 2, g] = 1.0          # e2 -> a_h num
        numk[g, 2, 32 + g] = 1.0     # e2 -> a_xp num
        numk[g, 2, 96 + g] = 1.0
        numk[g, 3, 64 + g] = 1.0     # e3 -> a_hxp num
        numk[g, 3, 96 + g] = 1.0
    selg = np.zeros((NH, NH, 128), np.float32)   # [k, g, m]
    for g in range(NH):
        selg[g, g, :] = 1.0
    onescol8 = np.zeros((128, 2, 64), np.float32)
    onescol8[:, 0, 0] = 1.0    # slot0 (y)   -> SUM row 0
    onescol8[:, 1, 32] = 1.0   # slot1 (y^2) -> SUM row 32
    return dict(
        oneseg=oneseg.astype(BF),
        onescol8=onescol8.astype(E4),
        numk=numk.astype(BF),
        selg=selg.astype(BF),
        onesrow=_to_f32r(np.ones((1, 128), np.float32)),
        id128=np.eye(128, dtype=np.float32).astype(BF),
    )


def _vec16(v):
    return np.ascontiguousarray(np.asarray(v, np.float32).reshape(NKH, 128).T)


def prepare_in_maps(h_prev, x, W_proj, b_proj, W_q, b_q, W_o, b_o, W_g, b_g,
                    gamma, beta):
    def _pack(wT):
        # [K, M] -> [m, p, k, c] contiguous (per-partition 8KB chunks)
        K_, M_ = wT.shape
        return np.ascontiguousarray(
            wT.reshape(K_ // 128, 128, M_ // 128, 128).transpose(2, 1, 0, 3))

    def _pack8(wT):
        # [K, M] -> [m, p, j, i, c]: fp8 pair-groups of 256 contraction rows
        K_, M_ = wT.shape
        w8 = (np.asarray(wT, np.float32) * WS).astype(E4)
        return np.ascontiguousarray(
            w8.reshape(K_ // 256, 2, 128, M_ // 128, 128).transpose(3, 2, 0, 1, 4))

    Wg = np.asarray(W_g, np.float32)
    shared = {
        "wp": _pack(np.asarray(W_proj, np.float32).T).astype(BF),
        "wq8": _pack8(np.asarray(W_q, np.float32).T),
        "wg1": _pack((Wg[:, :H] - Wg[:, H:]).T).astype(BF),
        "wo": _pack(np.asarray(W_o, np.float32).T).astype(BF),
        "wg2": _pack(Wg[:, H:].T).astype(BF),
        "bp": _vec16(b_proj), "bq": _vec16(b_q), "bo": _vec16(b_o),
        "bg": _vec16(b_g), "gam": _vec16(gamma), "bet": _vec16(beta),
    }
    shared.update(_consts_np())
    h2 = np.asarray(h_prev, np.float32).reshape(B, H)
    x2 = np.asarray(x, np.float32)
    in_maps = []
    for c in range(NCORES):
        m = dict(shared)
        hc = np.ascontiguousarray(h2[c * BL:(c + 1) * BL].T)
        m["h"] = hc.astype(BF)
        m["h8"] = hc.astype(E4)
        m["x"] = np.ascontiguousarray(x2[c * BL:(c + 1) * BL].T).astype(BF)
        in_maps.append(m)
    return in_maps


def run_device(in_maps, **kw):
    nc = _get_nc()
    return run_bass_kernel_spmd(nc, in_maps, core_ids=list(range(NCORES)), **kw)


_RUNNER = None


def _get_runner():
    """Custom sharded runner: per-core tensors sharded on the core axis,
    replicated weights/consts transferred once (not 8x)."""
    global _RUNNER
    if _RUNNER is not None:
        return _RUNNER
    import jax
    from jax.sharding import Mesh, PartitionSpec, NamedSharding
    try:
        from jax import shard_map as _sm
        shard_map = _sm.shard_map if hasattr(_sm, "shard_map") else _sm
    except Exception:
        from jax.experimental.shard_map import shard_map
    from concourse.bass2jax import _bass_exec_p, partition_id_tensor, \
        install_neuronx_cc_hook
    install_neuronx_cc_hook()

    nc = _get_nc()
    pid_name = nc.partition_id_tensor.name if nc.partition_id_tensor else None
    in_names, out_names, out_avals = [], [], []
    for alloc in nc.m.functions[0].allocations:
        if not isinstance(alloc, mybir.MemoryLocationSet):
            continue
        name = alloc.memorylocations[0].name
        if alloc.kind == "ExternalInput" and name != pid_name:
            in_names.append(name)
        elif alloc.kind == "ExternalOutput":
            out_names.append(name)
            out_avals.append(jax.core.ShapedArray(
                tuple(alloc.tensor_shape), mybir.dt.np(alloc.dtype)))
    bind_names = in_names + out_names + ([pid_name] if pid_name else [])
    sharded_names = {"h", "h8", "x"}

    def _body_fn(*args):
        operands = list(args)
        operands.append(partition_id_tensor())
        return tuple(_bass_exec_p.bind(
            *operands,
            out_avals=tuple(out_avals),
            in_names=tuple(bind_names),
            out_names=tuple(out_names),
            lowering_input_output_aliases=(),
            sim_require_finite=True,
            sim_require_nnan=True,
            nc=nc,
        ))

    devices = jax.devices()[:NCORES]
    mesh = Mesh(np.asarray(devices), ("core",))
    Pc, Pr = PartitionSpec("core"), PartitionSpec()
    in_specs = tuple(Pc if n in sharded_names else Pr for n in in_names) \
        + (Pc,) * len(out_names)
    import inspect
    _smkw = {}
    try:
        _p = inspect.signature(shard_map).parameters
        _smkw["check_rep" if "check_rep" in _p else "check_vma"] = False
    except Exception:
        _smkw["check_rep"] = False
    fn = jax.jit(
        shard_map(_body_fn, mesh=mesh, in_specs=in_specs,
                  out_specs=(Pc,) * len(out_names), **_smkw),
        keep_unused=True)
    dev_zeros = [
        jax.device_put(
            np.zeros((NCORES * av.shape[0], *av.shape[1:]), av.dtype),
            NamedSharding(mesh, Pc))
        for av in out_avals
    ]
    _RUNNER = (fn, mesh, in_names, out_names, out_avals, sharded_names, dev_zeros)
    return _RUNNER


def run_device_fast(in_maps):
    fn, mesh, in_names, out_names, out_avals, sharded_names, dev_zeros = _get_runner()
    args = []
    for n in in_names:
        if n in sharded_names:
            args.append(np.concatenate([np.asarray(m[n]) for m in in_maps], axis=0))
        else:
            args.append(np.asarray(in_maps[0][n]))
    args.extend(dev_zeros)
    outs = fn(*args)
    return {name: np.asarray(outs[i]) for i, name in enumerate(out_names)}


def kernel(**inputs):
    in_maps = prepare_in_maps(**inputs)
    try:
        outs = run_device_fast(in_maps)
        big = outs["outT"].reshape(NCORES, H, BL)
        out = np.empty((B, H), np.float32)
        for c in range(NCORES):
            out[c * BL:(c + 1) * BL] = big[c].T.astype(np.float32)
    except Exception:
        res = run_device(in_maps)
        out = np.empty((B, H), np.float32)
        for c in range(NCORES):
            out[c * BL:(c + 1) * BL] = np.asarray(
                res.results[c]["outT"], np.float32).T
    return out.reshape(B, 1, H)


# revision 22
# speedup vs baseline: 1.1697x; 1.0828x over previous
"""AttnRNNCell Trainium2 kernel: 8-core data-parallel over batch.

Feature-major activations [H, B_local]; bf16 matmuls except the q GEMM,
which runs in fp8e4 DoubleRow (2x PE rate; softmax damps the quantization).
The h-only half of the gate GEMM (wg1 = Wg1-Wg2 acting on h) is hoisted into
the softmax/attn-combine window to keep the PE busy, accumulated to SBUF, and
re-injected into the gate PSUM via an identity-stationary matmul.

Gate algebra: sigmoid(Wg1 h + Wg2 attn_o + bg) == sigmoid((Wg1-Wg2) h +
Wg2 (attn_o + h) + bg), so the gate GEMM part2 consumes y = attn_o + h.
"""
import sys

sys.path.insert(0, "/opt/trn_rl_repo")

import numpy as np
import ml_dtypes

import concourse.bass as bass
import concourse.tile as tile
import concourse.mybir as mybir
from concourse.bass_utils import run_bass_kernel_spmd

F32 = mybir.dt.float32
F32R = mybir.dt.float32r
BF16 = mybir.dt.bfloat16
FP8 = mybir.dt.float8e4
AF = mybir.ActivationFunctionType
DR = mybir.MatmulPerfMode.DoubleRow
BF = ml_dtypes.bfloat16
E4 = ml_dtypes.float8_e4m3

B, IN, H, NH = 8192, 1024, 2048, 4
HD = H // NH
EPS = 1e-5
NCORES = 8
BL = B // NCORES          # 1024 batch rows per core
NB = BL // 512            # 2 N-slices of 512
NKH = H // 128            # 16 feature tiles for H-sized dims
NKI = IN // 128           # 8 feature tiles for IN
NJH = NKH // 2            # 8 fp8 pair-groups for H-sized contraction
SCALE = 1.0 / float(np.sqrt(np.float32(HD)))
WS = 64.0                 # fp8 weight pre-scale

_DMA_OPS = ("InstDMACopy", "InstDMATranspose", "InstDMAMemset")


def _to_f32r(a):
    u = np.ascontiguousarray(a, dtype=np.float32).view(np.uint32)
    r = (u + 0x7FF + ((u >> 12) & 1)) & np.uint32(0xFFFFF000)
    return r.view(np.float32)


def _legalize_sync(nc, wait_cap=1, upd_cap=1):
    """This container's walrus supports ~1 sync wait/update per engine
    instruction; hoist the excess onto adjacent NoOps (same engine)."""
    ctr = [0]

    def mknop(eng, waits, upds):
        ctr[0] += 1
        nop = mybir.InstNoOp(name=f"lsync-{ctr[0]}", ins=[], outs=[])
        nop.engine = eng
        nop.sync_info = mybir.SyncInfo(on_wait=list(waits), on_update=list(upds))
        return nop

    for fn in nc.m.functions:
        for blk in fn.blocks:
            out = []
            changed = False
            for ins in blk.instructions:
                si = getattr(ins, "sync_info", None)
                if si is None:
                    out.append(ins)
                    continue
                waits = list(si.on_wait or [])
                upds = list(si.on_update or [])
                pre, post = [], []
                while len(waits) > wait_cap:
                    pre.append(mknop(ins.engine, [waits.pop(0)], []))
                if ins.__class__.__name__ not in _DMA_OPS:
                    while len(upds) > upd_cap:
                        post.append(mknop(ins.engine, [], [upds.pop()]))
                if pre or post:
                    ins.sync_info = mybir.SyncInfo(on_wait=waits, on_update=upds)
                    changed = True
                out.extend(pre)
                out.append(ins)
                out.extend(post)
            if changed:
                try:
                    blk.instructions = out
                except Exception:
                    blk.instructions.clear()
                    blk.instructions.extend(out)


def declare_dram(nc):
    d = {}
    d["h"] = nc.dram_tensor("h", [H, BL], BF16, kind="ExternalInput")
    d["h8"] = nc.dram_tensor("h8", [H, BL], FP8, kind="ExternalInput")
    d["x"] = nc.dram_tensor("x", [IN, BL], BF16, kind="ExternalInput")
    d["wp"] = nc.dram_tensor("wp", [NKH, 128, NKI, 128], BF16, kind="ExternalInput")
    d["wq8"] = nc.dram_tensor("wq8", [NKH, 128, NJH, 2, 128], FP8,
                              kind="ExternalInput")
    d["wg1"] = nc.dram_tensor("wg1", [NKH, 128, NKH, 128], BF16, kind="ExternalInput")
    d["wo"] = nc.dram_tensor("wo", [NKH, 128, NKH, 128], BF16, kind="ExternalInput")
    d["wg2"] = nc.dram_tensor("wg2", [NKH, 128, NKH, 128], BF16, kind="ExternalInput")
    for n in ("bp", "bq", "bo", "bg", "gam", "bet"):
        d[n] = nc.dram_tensor(n, [128, NKH], F32, kind="ExternalInput")
    d["oneseg"] = nc.dram_tensor("oneseg", [128, NH, NH], BF16, kind="ExternalInput")
    d["onescol8"] = nc.dram_tensor("onescol8", [128, 2, 64], FP8,
                                   kind="ExternalInput")
    d["numk"] = nc.dram_tensor("numk", [NH, NH, 128], BF16, kind="ExternalInput")
    d["selg"] = nc.dram_tensor("selg", [NH, NH, 128], BF16, kind="ExternalInput")
    d["onesrow"] = nc.dram_tensor("onesrow", [1, 128], F32R, kind="ExternalInput")
    d["id128"] = nc.dram_tensor("id128", [128, 128], BF16, kind="ExternalInput")
    d["outT"] = nc.dram_tensor("outT", [H, BL], BF16, kind="ExternalOutput")
    return d


def build():
    nc = bass.Bass()
    d = declare_dram(nc)
    with tile.TileContext(nc) as tc:
        _body(nc, tc, d)
    _legalize_sync(nc)
    return nc


def _body(nc, tc, d, stop_after=None):
    def _cut_A():
        nc.sync.dma_start(out=d["outT"][0:128, :], in_=xp[:, 0, :])
        p_h8.release()
        p_xp.release()
        p_g1.release()
        p_out.release()
        p_tmp.release()
        p_ws.release()
        p_h.release()
        consts.release()
        ps_main.release()

    consts = tc.alloc_tile_pool(name="consts", bufs=1, side="left")
    p_h = tc.alloc_tile_pool(name="p_h", bufs=1, side="left")
    p_ws = tc.alloc_tile_pool(name="p_ws", bufs=2, side="left")
    p_tmp = tc.alloc_tile_pool(name="p_tmp", bufs=6, side="left")
    p_out = tc.alloc_tile_pool(name="p_out", bufs=2, side="left")
    p_g1 = tc.alloc_tile_pool(name="p_g1", bufs=1, side="left")
    ps_main = tc.alloc_tile_pool(name="ps_main", bufs=2, space="PSUM", side="left")

    wp_t = {}

    def fetch_wp(m):
        if m < NKH:
            w = p_ws.tile([128, NKI, 128], BF16, tag="wr", bufs=3)
            nc.sync.dma_start(out=w[:, :, :], in_=d["wp"][m, :, :, :])
            wp_t[m] = w

    for m in range(2):
        fetch_wp(m)

    # ---- consts (after first weights in the SP DMA queue) ----
    bt = {}
    for n in ("bp", "bq", "bo", "bg", "gam", "bet"):
        bt[n] = consts.tile([128, NKH], F32, tag='bias_' + n, name='bias_' + n)
        nc.sync.dma_start(out=bt[n], in_=d[n][:, :])
    oneseg = consts.tile([128, NH, NH], BF16)
    nc.sync.dma_start(out=oneseg, in_=d["oneseg"][:, :, :])
    onescol8 = consts.tile([128, 2, 64], FP8)
    nc.sync.dma_start(out=onescol8, in_=d["onescol8"][:, :, :])
    numk = consts.tile([NH, NH, 128], BF16)
    nc.sync.dma_start(out=numk, in_=d["numk"][:, :, :])
    selg = consts.tile([NH, NH, 128], BF16)
    nc.sync.dma_start(out=selg, in_=d["selg"][:, :, :])
    onesrow = consts.tile([1, 128], F32R)
    nc.sync.dma_start(out=onesrow, in_=d["onesrow"][:, :])
    id128 = consts.tile([128, 128], BF16)
    nc.sync.dma_start(out=id128, in_=d["id128"][:, :])
    epst = consts.tile([1, 1], F32)
    nc.vector.memset(epst, EPS)

    # PE p-state warmup during the initial DMA wait: dep-free junk matmuls
    wrm = consts.tile([128, 640], BF16, tag="wrm")
    nc.vector.memset(wrm, 0.0)
    wpo = ps_main.tile([128, 512], F32, tag="po")
    for r in range(6):
        nc.tensor.matmul(wpo[:, :], wrm[:, 0:128], wrm[:, 128:640],
                         start=(r == 0), stop=(r == 5))

    # ---- x first (A needs it immediately) ----
    p_xp = tc.alloc_tile_pool(name="p_xp", bufs=1, side="right")
    p_h8 = tc.alloc_tile_pool(name="p_h8", bufs=1, side="right")
    p_x = tc.alloc_tile_pool(name="p_x", bufs=1, side="right")
    xT = p_x.tile([128, NKI, BL], BF16)
    for k in range(NKI):
        nc.gpsimd.dma_start(out=xT[:, k, :], in_=d["x"][k * 128:(k + 1) * 128, :])

    hT = p_h.tile([128, NKH, BL], BF16)
    h8 = p_h8.tile([128, NKH, BL], FP8)

    # ---- A: xp = Wp.T @ x + bp  (bf16) ----
    xp = p_xp.tile([128, NKH, BL], BF16)
    for m in range(NKH):
        fetch_wp(m + 2)
        if m == 2:      # h8 needed by B; hT by B's products (parallel queue)
            for k in range(NKH):
                nc.gpsimd.dma_start(out=h8[:, k, :],
                                    in_=d["h8"][k * 128:(k + 1) * 128, :])
        if m == 6:
            for k in range(NKH):
                nc.gpsimd.dma_start(out=hT[:, k, :],
                                    in_=d["h"][k * 128:(k + 1) * 128, :])
        w = wp_t.pop(m)
        po = ps_main.tile([128, BL], F32, tag="po")
        for bs in range(NB):
            sl = slice(bs * 512, bs * 512 + 512)
            for k in range(NKI):
                nc.tensor.matmul(po[:, sl], w[:, k, :], xT[:, k, sl],
                                 start=(k == 0), stop=(k == NKI - 1))
        nc.scalar.activation(xp[:, m, :], po[:, :], AF.Identity,
                             bias=bt["bp"][:, m:m + 1], scale=1.0)
    p_x.release()
    if stop_after == "A":
        _cut_A()
        return

    wq_t = {}

    def fetch_wq(m):
        if m < NKH:
            w = p_ws.tile([128, NJH, 2, 128], FP8, tag="wq")
            nc.sync.dma_start(out=w[:, :, :, :], in_=d["wq8"][m, :, :, :, :])
            wq_t[m] = w

    for m in range(2):
        fetch_wq(m)

    # ---- B: q GEMM (fp8 DoubleRow) + products + score reductions ----
    ps_sB = tc.alloc_tile_pool(name="ps_sB", bufs=1, space="PSUM", side="left")
    S = ps_sB.tile([128, BL], F32, tag="sps")   # rows 0-3: s0, 32-35: s1, 64-67: s3
    pend = []
    emit_ctr = [0]

    def emit_scores(g, p0, p1, p3):
        st = emit_ctr[0] == 0
        sp = emit_ctr[0] == NKH - 1
        emit_ctr[0] += 1
        for bs in range(NB):
            sl = slice(bs * 512, bs * 512 + 512)
            nc.tensor.matmul(S[0:4, sl], oneseg[:, g, :], p0[:, sl], start=st, stop=sp)
            nc.tensor.matmul(S[32:36, sl], oneseg[:, g, :], p1[:, sl], start=st, stop=sp)
            nc.tensor.matmul(S[64:68, sl], oneseg[:, g, :], p3[:, sl], start=st, stop=sp)

    for m in range(NKH):
        fetch_wq(m + 2)
        g = m // NH
        w = wq_t.pop(m)
        po = ps_main.tile([128, BL], F32, tag="po")
        for bs in range(NB):
            sl = slice(bs * 512, bs * 512 + 512)
            for j in range(NJH):
                nc.tensor.matmul(po[:, sl], w[:, j, :, :],
                                 h8[:, 2 * j:2 * j + 2, sl],
                                 start=(j == 0), stop=(j == NJH - 1),
                                 perf_mode=DR)
        qm = p_tmp.tile([128, BL], BF16, tag="tb", name="qm")
        nc.scalar.activation(qm[:, :], po[:, :], AF.Identity,
                             bias=bt["bq"][:, m:m + 1], scale=1.0 / WS)
        p0 = p_tmp.tile([128, BL], BF16, tag="tb", name="p0")
        p1 = p_tmp.tile([128, BL], BF16, tag="tb", name="p1")
        p3 = p_tmp.tile([128, BL], BF16, tag="tb", name="p3")
        nc.vector.tensor_mul(p0[:, :], qm[:, :], hT[:, m, :])
        nc.vector.tensor_mul(p1[:, :], qm[:, :], xp[:, m, :])
        nc.vector.tensor_mul(p3[:, :], p0[:, :], xp[:, m, :])
        pend.append((m // NH, p0, p1, p3))
        if len(pend) == 2:
            emit_scores(*pend.pop(0))
    while pend:
        emit_scores(*pend.pop(0))
    p_h8.release()
    if stop_after == "B":
        nc.sync.dma_start(out=d["outT"][0:128, :], in_=xp[:, 0, :])
        ps_sB.release()
        p_xp.release()
        p_g1.release()
        p_out.release()
        p_tmp.release()
        p_ws.release()
        p_h.release()
        consts.release()
        ps_main.release()
        return

    # ---- G1 blocks: g1[m] = (Wg1-Wg2).T @ h + bg, hoisted PE filler ----
    g1 = p_g1.tile([128, NKH, BL], BF16)
    g1_next = [0]
    wg1_t = {}

    def fetch_wg1(m):
        if m < NKH:
            w = p_ws.tile([128, NKH, 128], BF16, tag="wg1")
            nc.sync.dma_start(out=w[:, :, :], in_=d["wg1"][m, :, :, :])
            wg1_t[m] = w

    fetch_wg1(0)

    def g1_block():
        m = g1_next[0]
        if m >= NKH:
            return
        g1_next[0] += 1
        fetch_wg1(m + 1)
        w = wg1_t.pop(m)
        po = ps_main.tile([128, BL], F32, tag="po")
        for bs in range(NB):
            sl = slice(bs * 512, bs * 512 + 512)
            for k in range(NKH):
                nc.tensor.matmul(po[:, sl], w[:, k, :], hT[:, k, sl],
                                 start=(k == 0), stop=(k == NKH - 1))
        nc.scalar.activation(g1[:, m, :], po[:, :], AF.Identity,
                             bias=bt["bg"][:, m:m + 1], scale=1.0)

    g1_block()

    # ---- C: softmax coefficients ----
    p_smA = tc.alloc_tile_pool(name="p_smA", bufs=1, side="right")
    E0 = p_smA.tile([4, BL], BF16)
    E1 = p_smA.tile([4, BL], BF16)
    E2 = p_smA.tile([4, BL], BF16)
    E3 = p_smA.tile([4, BL], BF16)
    nc.scalar.activation(E0[:, :], S[0:4, :], AF.Exp, scale=SCALE)
    nc.scalar.activation(E1[:, :], S[32:36, :], AF.Exp, scale=SCALE)
    nc.scalar.activation(E3[:, :], S[64:68, :], AF.Exp, scale=SCALE)
    nc.vector.tensor_mul(E2[:, :], E0[:, :], E1[:, :])   # exp(s0+s1) == e0*e1
    NUM = ps_sB.tile([128, BL], F32, tag="sps")
    for qi, Eq in enumerate((E0, E1, E2, E3)):
        for bs in range(NB):
            sl = slice(bs * 512, bs * 512 + 512)
            nc.tensor.matmul(NUM[:, sl], numk[:, qi, :], Eq[:, sl],
                             start=(qi == 0), stop=(qi == 3))
    R = p_smA.tile([4, BL], F32)
    nc.vector.reciprocal(R[:, :], NUM[96:100, :])
    Ah = p_smA.tile([4, BL], BF16)
    Ax = p_smA.tile([4, BL], BF16)
    Az = p_smA.tile([4, BL], BF16)
    nc.vector.tensor_mul(Ah[:, :], NUM[0:4, :], R[:, :])
    nc.vector.tensor_mul(Ax[:, :], NUM[32:36, :], R[:, :])
    nc.vector.tensor_mul(Az[:, :], NUM[64:68, :], R[:, :])
    ps_sB.release()

    g1_block()
    g1_block()

    # ---- D1: attn combine (per head, per 512-slice; bf16 coefficients) ----
    p_attn = tc.alloc_tile_pool(name="p_attn", bufs=1, side="left")
    p_cb = tc.alloc_tile_pool(name="p_cb", bufs=2, side="right")
    ps_cD = tc.alloc_tile_pool(name="ps_cD", bufs=1, space="PSUM", side="left")
    # right-stack release order after D1: p_cb, p_smA, p_xp (LIFO)
    attn = p_attn.tile([128, NKH, BL], BF16)
    for g in range(NH):
        for bs in range(NB):
            sl = slice(bs * 512, bs * 512 + 512)
            CH = ps_cD.tile([128, 512], F32, tag="CH")
            CX = ps_cD.tile([128, 512], F32, tag="CX")
            CZ = ps_cD.tile([128, 512], F32, tag="CZ")
            nc.tensor.matmul(CH[:, :], selg[:, g, :], Ah[:, sl], start=True, stop=True)
            nc.tensor.matmul(CX[:, :], selg[:, g, :], Ax[:, sl], start=True, stop=True)
            nc.tensor.matmul(CZ[:, :], selg[:, g, :], Az[:, sl], start=True, stop=True)
            CHb = p_cb.tile([128, 512], BF16, tag="CHb", name="CHb")
            CXb = p_cb.tile([128, 512], BF16, tag="CXb", name="CXb")
            CZb = p_cb.tile([128, 512], BF16, tag="CZb", name="CZb")
            nc.scalar.activation(CHb[:, :], CH[:, :], AF.Identity, scale=1.0)
            nc.scalar.activation(CXb[:, :], CX[:, :], AF.Identity, scale=1.0)
            nc.scalar.activation(CZb[:, :], CZ[:, :], AF.Identity, scale=1.0)
            g1_block()
            for m in range(g * NH, (g + 1) * NH):
                t0 = p_tmp.tile([128, 512], BF16, tag="tf", name="t0", bufs=3)
                t1 = p_tmp.tile([128, 512], BF16, tag="tf", name="t1", bufs=3)
                nc.vector.tensor_mul(t0[:, :], xp[:, m, sl], CZb[:, :])
                nc.vector.tensor_add(t0[:, :], t0[:, :], CHb[:, :])
                nc.vector.tensor_mul(t0[:, :], t0[:, :], hT[:, m, sl])
                nc.vector.tensor_mul(t1[:, :], xp[:, m, sl], CXb[:, :])
                nc.vector.tensor_add(attn[:, m, sl], t0[:, :], t1[:, :])
    ps_cD.release()
    p_cb.release()
    p_smA.release()
    p_xp.release()

    while g1_next[0] < NKH:
        g1_block()
    if stop_after == "D1":
        nc.sync.dma_start(out=d["outT"][0:128, :], in_=attn[:, 0, :])
        p_attn.release()
        p_g1.release()
        p_out.release()
        p_tmp.release()
        p_ws.release()
        p_h.release()
        consts.release()
        ps_main.release()
        return

    # ---- D2 + F fused per m: attn_o = gelu(Wo attn + bo); y = attn_o + h;
    #      LN sums ----
    p_wb = tc.alloc_tile_pool(name="p_wb", bufs=3, side="right")
    p_y = tc.alloc_tile_pool(name="p_y", bufs=1, side="left")
    ps_sF = tc.alloc_tile_pool(name="ps_sF", bufs=1, space="PSUM", side="left")
    y = p_y.tile([128, NKH, BL], BF16)
    SUM = ps_sF.tile([128, BL], F32, tag="sum")

    wo_t = {}
    wg2_t = {}

    def fetch_wo(m):
        if m < NKH:
            w = p_wb.tile([128, NKH, 128], BF16, tag="wb")
            nc.sync.dma_start(out=w[:, :, :], in_=d["wo"][m, :, :, :])
            wo_t[m] = w

    def fetch_wg2(m):
        if m < NKH:
            w = p_wb.tile([128, NKH, 128], BF16, tag="wb")
            nc.sync.dma_start(out=w[:, :, :], in_=d["wg2"][m, :, :, :])
            wg2_t[m] = w

    for m in range(2):
        fetch_wo(m)

    for m in range(NKH):
        fetch_wo(m + 2)
        if m == 14:
            fetch_wg2(0)
        if m == 15:
            fetch_wg2(1)
        w = wo_t.pop(m)
        po = ps_main.tile([128, BL], F32, tag="po")
        for bs in range(NB):
            sl = slice(bs * 512, bs * 512 + 512)
            for k in range(NKH):
                nc.tensor.matmul(po[:, sl], w[:, k, :], attn[:, k, sl],
                                 start=(k == 0), stop=(k == NKH - 1))
        ao_t = p_tmp.tile([128, BL], BF16, tag="tb", name="ao")
        nc.scalar.activation(ao_t[:, :], po[:, :], AF.Gelu,
                             bias=bt["bo"][:, m:m + 1], scale=1.0)
        nc.vector.tensor_add(y[:, m, :], ao_t[:, :], hT[:, m, :])
        yp8 = p_tmp.tile([128, 2, BL], FP8, tag="p8", name="yp8", bufs=3)
        nc.scalar.activation(yp8[:, 0, :], y[:, m, :], AF.Identity, scale=1.0)
        nc.vector.tensor_mul(yp8[:, 1, :], y[:, m, :], y[:, m, :])
        st, sp = (m == 0), (m == NKH - 1)
        for bs in range(NB):
            sl = slice(bs * 512, bs * 512 + 512)
            nc.tensor.matmul(SUM[0:64, sl], onescol8[:, :, :], yp8[:, :, sl],
                             start=st, stop=sp, perf_mode=DR)

    if stop_after == "D2":
        nc.sync.dma_start(out=d["outT"][0:128, :], in_=y[:, 0, :])
        ps_sF.release()
        p_wb.release()
        p_y.release()
        p_attn.release()
        p_g1.release()
        p_out.release()
        p_tmp.release()
        p_ws.release()
        p_h.release()
        consts.release()
        ps_main.release()
        return

    # ---- G: mu / rstd rows + PE broadcast ----
    p_smB = tc.alloc_tile_pool(name="p_smB", bufs=1, side="right")
    MUr = p_smB.tile([1, BL], F32R)
    MSQ = p_smB.tile([1, BL], F32)
    nc.vector.tensor_scalar(out=MUr[:, :], in0=SUM[0:1, :], scalar1=1.0 / H,
                            scalar2=None, op0=mybir.AluOpType.mult)
    nc.vector.tensor_scalar(out=MSQ[:, :], in0=SUM[32:33, :], scalar1=1.0 / H,
                            scalar2=None, op0=mybir.AluOpType.mult)
    MUf = MUr.bitcast(F32)
    MU2t = p_tmp.tile([128, BL], F32, tag="tf32", name="mu2", bufs=2)
    MU2 = MU2t[0:1, :]
    nc.vector.tensor_mul(MU2, MUf[:, :], MUf[:, :])
    nc.vector.tensor_sub(MSQ[:, :], MSQ[:, :], MU2)
    nc.scalar.activation(MU2, MSQ[:, :], AF.Sqrt, bias=epst[:, 0:1], scale=1.0)
    nc.vector.reciprocal(MSQ[:, :], MU2)
    RSTr = p_smB.tile([1, BL], F32R)
    nc.vector.tensor_copy(RSTr[:, :], MSQ[:, :])
    ps_sF.release()

    # first two gate-part2 PSUMs before the broadcast matmuls (PE filler)
    def po2_block(m):
        w = wg2_t.pop(m)
        po = ps_main.tile([128, BL], F32, tag="po")
        late = m >= NKH - 2   # inject g1 on the PE for the tail iterations
        for bs in range(NB):
            sl = slice(bs * 512, bs * 512 + 512)
            for k in range(NKH):
                nc.tensor.matmul(po[:, sl], w[:, k, :], y[:, k, sl],
                                 start=(k == 0),
                                 stop=(k == NKH - 1 and not late))
            if late:
                nc.tensor.matmul(po[:, sl], id128[:, :], g1[:, m, sl],
                                 start=False, stop=True)
        return po

    po2_cache = {}
    po2_cache[0] = po2_block(0)

    ps_gh = tc.alloc_tile_pool(name="ps_gh", bufs=1, space="PSUM", side="left")
    MUB = ps_gh.tile([128, BL], F32, tag="mub")
    RSB = ps_gh.tile([128, BL], F32, tag="rsb")
    for bs in range(NB):
        sl = slice(bs * 512, bs * 512 + 512)
        nc.tensor.matmul(MUB[:, sl], onesrow[:, :], MUr[:, sl], start=True, stop=True)
        nc.tensor.matmul(RSB[:, sl], onesrow[:, :], RSTr[:, sl], start=True, stop=True)

    # ---- H: gate sigmoid + normalize + blend + out (fused per m) ----
    for m in range(NKH):
        fetch_wg2(m + 2)
        po = po2_cache.pop(m) if m in po2_cache else po2_block(m)
        gm = p_tmp.tile([128, BL], BF16, tag="tb", name="gm")
        late = m >= NKH - 2
        if not late:
            pre = p_tmp.tile([128, BL], BF16, tag="tb", name="pre")
            nc.vector.tensor_add(pre[:, :], po[:, :], g1[:, m, :])
            nc.scalar.activation(gm[:, :], pre[:, :], AF.Sigmoid, scale=1.0)
        t0 = p_tmp.tile([128, BL], F32, tag="tf32", name="n0", bufs=2)
        nc.vector.tensor_sub(t0[:, :], y[:, m, :], MUB[:, :])
        nc.vector.tensor_mul(t0[:, :], t0[:, :], RSB[:, :])
        nc.vector.tensor_scalar(out=t0[:, :], in0=t0[:, :],
                                scalar1=bt["gam"][:, m:m + 1],
                                scalar2=bt["bet"][:, m:m + 1],
                                op0=mybir.AluOpType.mult, op1=mybir.AluOpType.add)
        t1 = p_tmp.tile([128, BL], BF16, tag="tb", name="n1")
        ot = p_out.tile([128, BL], BF16, tag="ot")
        if late:
            # tail iterations: 512-col halves pipeline sigmoid/blend/DMA
            for bs in range(NB):
                sl = slice(bs * 512, bs * 512 + 512)
                nc.scalar.activation(gm[:, sl], po[:, sl], AF.Sigmoid, scale=1.0)
                nc.vector.tensor_sub(t1[:, sl], t0[:, sl], hT[:, m, sl])
                nc.vector.tensor_mul(t1[:, sl], t1[:, sl], gm[:, sl])
                nc.vector.tensor_add(ot[:, sl], t1[:, sl], hT[:, m, sl])
                nc.sync.dma_start(out=d["outT"][m * 128:(m + 1) * 128, sl],
                                  in_=ot[:, sl])
        else:
            nc.gpsimd.tensor_sub(t1[:, :], t0[:, :], hT[:, m, :])
            nc.gpsimd.tensor_mul(t1[:, :], t1[:, :], gm[:, :])
            nc.vector.tensor_add(ot[:, :], t1[:, :], hT[:, m, :])
            nc.sync.dma_start(out=d["outT"][m * 128:(m + 1) * 128, :],
                              in_=ot[:, :])

    p_smB.release()
    p_wb.release()
    p_y.release()
    p_attn.release()
    p_g1.release()
    p_out.release()
    p_tmp.release()
    p_ws.release()
    p_h.release()
    consts.release()
    ps_gh.release()
    ps_main.release()


_NC = None


def _get_nc():
    global _NC
    if _NC is None:
        _NC = build()
    return _NC


def _consts_np():
    oneseg = np.zeros((128, NH, NH), np.float32)
    for g in range(NH):
        oneseg[:, g, g] = 1.0
    numk = np.zeros((NH, NH, 128), np.float32)   # [k=g, q, m]
    for g in range(NH):
        numk[g, 0, g] = 1.0          # e0 -> a_h num
        numk[g, 0, 96 + g] = 1.0     # e0 -> denom
        numk[g, 1, 32 + g] = 1.0     # e1 -> a_xp num
        numk[g, 1, 96 + g] = 1.0
        numk[g, 2, g] = 1.0          # e2 -> a_h num
        numk[g, 2, 32 + g] = 1.0     # e2 -> a_xp num
        numk[g, 2, 96 + g] = 1.0
        numk[g, 3, 64 + g] = 1.0     # e3 -> a_hxp num
        numk[g, 3, 96 + g] = 1.0
    selg = np.zeros((NH, NH, 128), np.float32)   # [k, g, m]
    for g in range(NH):
        selg[g, g, :] = 1.0
    onescol8 = np.zeros((128, 2, 64), np.float32)
    onescol8[:, 0, 0] = 1.0    # slot0 (y)   -> SUM row 0
    onescol8[:, 1, 32] = 1.0   # slot1 (y^2) -> SUM row 32
    return dict(
        oneseg=oneseg.astype(BF),
        onescol8=onescol8.astype(E4),
        numk=numk.astype(BF),
        selg=selg.astype(BF),
        onesrow=_to_f32r(np.ones((1, 128), np.float32)),
        id128=np.eye(128, dtype=np.float32).astype(BF),
    )


def _vec16(v):
    return np.ascontiguousarray(np.asarray(v, np.float32).reshape(NKH, 128).T)


def prepare_in_maps(h_prev, x, W_proj, b_proj, W_q, b_q, W_o, b_o, W_g, b_g,
                    gamma, beta):
    def _pack(wT):
        # [K, M] -> [m, p, k, c] contiguous (per-partition 8KB chunks)
        K_, M_ = wT.shape
        return np.ascontiguousarray(
            wT.reshape(K_ // 128, 128, M_ // 128, 128).transpose(2, 1, 0, 3))

    def _pack8(wT):
        # [K, M] -> [m, p, j, i, c]: fp8 pair-groups of 256 contraction rows
        K_, M_ = wT.shape
        w8 = (np.asarray(wT, np.float32) * WS).astype(E4)
        return np.ascontiguousarray(
            w8.reshape(K_ // 256, 2, 128, M_ // 128, 128).transpose(3, 2, 0, 1, 4))

    Wg = np.asarray(W_g, np.float32)
    shared = {
        "wp": _pack(np.asarray(W_proj, np.float32).T).astype(BF),
        "wq8": _pack8(np.asarray(W_q, np.float32).T),
        "wg1": _pack((Wg[:, :H] - Wg[:, H:]).T).astype(BF),
        "wo": _pack(np.asarray(W_o, np.float32).T).astype(BF),
        "wg2": _pack(Wg[:, H:].T).astype(BF),
        "bp": _vec16(b_proj), "bq": _vec16(b_q), "bo": _vec16(b_o),
        "bg": _vec16(b_g), "gam": _vec16(gamma), "bet": _vec16(beta),
    }
    shared.update(_consts_np())
    h2 = np.asarray(h_prev, np.float32).reshape(B, H)
    x2 = np.asarray(x, np.float32)
    in_maps = []
    for c in range(NCORES):
        m = dict(shared)
        hc = np.ascontiguousarray(h2[c * BL:(c + 1) * BL].T)
        m["h"] = hc.astype(BF)
        m["h8"] = hc.astype(E4)
        m["x"] = np.ascontiguousarray(x2[c * BL:(c + 1) * BL].T).astype(BF)
        in_maps.append(m)
    return in_maps


def run_device(in_maps, **kw):
    nc = _get_nc()
    return run_bass_kernel_spmd(nc, in_maps, core_ids=list(range(NCORES)), **kw)


_RUNNER = None


def _get_runner():
    """Custom sharded runner: per-core tensors sharded on the core axis,
    replicated weights/consts transferred once (not 8x)."""
    global _RUNNER
    if _RUNNER is not None:
        return _RUNNER
    import jax
    from jax.sharding import Mesh, PartitionSpec, NamedSharding
    try:
        from jax import shard_map as _sm
        shard_map = _sm.shard_map if hasattr(_sm, "shard_map") else _sm
    except Exception:
        from jax.experimental.shard_map import shard_map
    from concourse.bass2jax import _bass_exec_p, partition_id_tensor, \
        install_neuronx_cc_hook
    install_neuronx_cc_hook()

    nc = _get_nc()
    pid_name = nc.partition_id_tensor.name if nc.partition_id_tensor else None
    in_names, out_names, out_avals = [], [], []
    for alloc in nc.m.functions[0].allocations:
        if not isinstance(alloc, mybir.MemoryLocationSet):
            continue
        name = alloc.memorylocations[0].name
        if alloc.kind == "ExternalInput" and name != pid_name:
            in_names.append(name)
        elif alloc.kind == "ExternalOutput":
            out_names.append(name)
            out_avals.append(jax.core.ShapedArray(
                tuple(alloc.tensor_shape), mybir.dt.np(alloc.dtype)))
    bind_names = in_names + out_names + ([pid_name] if pid_name else [])
    sharded_names = {"h", "h8", "x"}

    def _body_fn(*args):
        operands = list(args)
        operands.append(partition_id_tensor())
        return tuple(_bass_exec_p.bind(
            *operands,
            out_avals=tuple(out_avals),
            in_names=tuple(bind_names),
            out_names=tuple(out_names),
            lowering_input_output_aliases=(),
            sim_require_finite=True,
            sim_require_nnan=True,
            nc=nc,
        ))

    devices = jax.devices()[:NCORES]
    mesh = Mesh(np.asarray(devices), ("core",))
    Pc, Pr = PartitionSpec("core"), PartitionSpec()
    in_specs = tuple(Pc if n in sharded_names else Pr for n in in_names) \
        + (Pc,) * len(out_names)
    import inspect
    _smkw = {}
    try:
        _p = inspect.signature(shard_map).parameters
        _smkw["check_rep" if "check_rep" in _p else "check_vma"] = False
    except Exception:
        _smkw["check_rep"] = False
    fn = jax.jit(
        shard_map(_body_fn, mesh=mesh, in_specs=in_specs,
                  out_specs=(Pc,) * len(out_names), **_smkw),
        keep_unused=True)
    dev_zeros = [
        jax.device_put(
            np.zeros((NCORES * av.shape[0], *av.shape[1:]), av.dtype),
            NamedSharding(mesh, Pc))
        for av in out_avals
    ]
    _RUNNER = (fn, mesh, in_names, out_names, out_avals, sharded_names, dev_zeros)
    return _RUNNER


def run_device_fast(in_maps):
    fn, mesh, in_names, out_names, out_avals, sharded_names, dev_zeros = _get_runner()
    args = []
    for n in in_names:
        if n in sharded_names:
            args.append(np.concatenate([np.asarray(m[n]) for m in in_maps], axis=0))
        else:
            args.append(np.asarray(in_maps[0][n]))
    args.extend(dev_zeros)
    outs = fn(*args)
    return {name: np.asarray(outs[i]) for i, name in enumerate(out_names)}


def kernel(**inputs):
    in_maps = prepare_in_maps(**inputs)
    try:
        outs = run_device_fast(in_maps)
        big = outs["outT"].reshape(NCORES, H, BL)
        out = np.empty((B, H), np.float32)
        for c in range(NCORES):
            out[c * BL:(c + 1) * BL] = big[c].T.astype(np.float32)
    except Exception:
        res = run_device(in_maps)
        out = np.empty((B, H), np.float32)
        for c in range(NCORES):
            out[c * BL:(c + 1) * BL] = np.asarray(
                res.results[c]["outT"], np.float32).T
    return out.reshape(B, 1, H)
